# revision 37
# baseline (speedup 1.0000x reference)
"""DeepSeekV3 MLA attention prefill kernel for 8 Trainium2 NeuronCores.

Sharding: sequence-parallel for the low-rank input projections (q_a / kv_a),
AllGather of the shared latents, tensor-parallel over heads (4 heads/core) for
q_b / kv_b decompression and attention, AllGather of attention outputs, and
output-column-parallel o_proj (each core produces a disjoint 512-column slice
of the output, concatenated on host).

Schedule notes (final):
- kv_a runs first so AG_kv is the first collective (absorbs the ncfw
  pickup + init-barrier latency); its latents DMA in during q_a
- AG_q is split into two pipelined halves (rt-major transposed writes) so
  q_b can start contracting on the first half while the second transfers
- kv_b decompression emitted between q_a and q_b, filling the AG_q window
- input/weight DMAs spread across sync/scalar/gpsimd issue queues with
  host-side DMA-contiguous weight layouts
- attention software-pipelined 3 key-tiles ahead of the exp chain; softmax
  finalize (den broadcast / fast-approx reciprocal / g2 write) deferred
  behind the next block's matmuls so the PE never drains at boundaries
- o_proj: g2 reads prefetched per head-group as its AllGather lands,
  deep PSUM buffering, SBUF accumulation across head-groups
"""

import sys

sys.path.insert(0, "/opt/trn_rl_repo")

import numpy as np
import ml_dtypes

import concourse.bass as bass  # noqa: F401
import concourse.mybir as mybir
from concourse import bacc
from concourse.bass import ds, ts
from concourse.tile import TileContext
from concourse.bass_utils import run_bass_kernel_spmd
from contextlib import ExitStack

F = mybir.dt.float32
BF = mybir.dt.bfloat16
R = mybir.dt.float32r
I32 = mybir.dt.int32
AF = mybir.ActivationFunctionType
ALU = mybir.AluOpType

NCORES = 8
B, S, H = 1, 2048, 4096
N_HEADS = 32
HPC = N_HEADS // NCORES          # heads per core = 4
SL = S // NCORES                 # sequence rows per core = 256
QR, KR = 1536, 512
DR, DN, DV = 64, 128, 128
QD = DN + DR                     # 192
SCALE = QD ** -0.5
EPS = 1e-6
THETA = 10000.0
TWO_PI = float(2.0 * np.pi)
MCOLS = H // NCORES              # output columns per core = 512

LAST_RESULT = None               # test harness reads exec_time_ns from here
_CACHED_NC = None
_UID = [0]


def _uid():
    _UID[0] += 1
    return _UID[0]


def _emit_range_reduce(nc, pool, t_ap, width):
    """In-place wrap t_ap (f32, [128, width]) to [-pi, pi]. f32->i32 copy
    rounds to nearest (verified on HW)."""
    tn = pool.tile([128, width], F, tag=f"rr_f_{width}", name=f"rrf{_uid()}")
    ti = pool.tile([128, width], I32, tag=f"rr_i_{width}", name=f"rri{_uid()}")
    nc.vector.tensor_scalar_mul(tn[:], t_ap, 1.0 / TWO_PI)
    nc.vector.tensor_copy(ti[:], tn[:])
    nc.vector.tensor_copy(tn[:], ti[:])
    nc.vector.tensor_scalar_mul(tn[:], tn[:], -TWO_PI)
    nc.vector.tensor_tensor(t_ap, t_ap, tn[:], ALU.add)


def _build_program():
    nc = bacc.Bacc(None, target_bir_lowering=False, num_devices=NCORES)

    # ---------------- DRAM declarations ----------------
    xt_d = nc.dram_tensor("xt_d", [128, 32, SL], BF, kind="ExternalInput")
    qaw_d = nc.dram_tensor("qaw_d", [12, 128, 8, 512], BF, kind="ExternalInput")
    kvaw_d = nc.dram_tensor("kvaw_d", [8, 128, 8, 288], BF, kind="ExternalInput")
    qbwT = nc.dram_tensor("qbwT", [QR, 1024], BF, kind="ExternalInput")
    kvbwT = nc.dram_tensor("kvbwT", [KR, 1024], BF, kind="ExternalInput")
    ow_d = nc.dram_tensor("ow_d", [HPC, 128, NCORES, MCOLS], BF, kind="ExternalInput")
    pos_all = nc.dram_tensor("pos_all", [1, S], I32, kind="ExternalInput")
    pos_loc = nc.dram_tensor("pos_loc", [SL], I32, kind="ExternalInput")
    ident_d = nc.dram_tensor("ident", [128, 128], F, kind="ExternalInput")
    triu_d = nc.dram_tensor("triu", [128, 128], F, kind="ExternalInput")
    if32_d = nc.dram_tensor("if32", [128, 32], F, kind="ExternalInput")
    if128_d = nc.dram_tensor("if128", [128, 1], F, kind="ExternalInput")
    out_d = nc.dram_tensor("out", [MCOLS, S], F, kind="ExternalOutput")

    g1kv_src = nc.dram_tensor("g1kv_src", [KR + DR, SL], BF)
    g1kv = nc.dram_tensor("g1kv", [NCORES, KR + DR, SL], BF, addr_space="Shared")
    QH = QR // 2
    g1q_srcA = nc.dram_tensor("g1q_srcA", [QH, SL], BF)
    g1qA = nc.dram_tensor("g1qA", [NCORES, QH, SL], BF, addr_space="Shared")
    g1q_srcB = nc.dram_tensor("g1q_srcB", [QH, SL], BF)
    g1qB = nc.dram_tensor("g1qB", [NCORES, QH, SL], BF, addr_space="Shared")
    warm_src = nc.dram_tensor("warm_src", [1, 128], BF)
    warm_dst = nc.dram_tensor("warm_dst", [NCORES, 1, 128], BF, addr_space="Shared")
    g2_src = [nc.dram_tensor(f"g2src{h}", [DV, S], BF) for h in range(HPC)]
    g2 = [
        nc.dram_tensor(f"g2_{h}", [NCORES, DV, S], BF, addr_space="Shared")
        for h in range(HPC)
    ]
    RG = [list(range(NCORES))]
    NKT = H // 128  # 32 k-tiles over the model dim
    NR = QR // 128  # 12 k-tiles over q_lora_rank
    NKR = KR // 128  # 4 k-tiles over kv_lora_rank

    with TileContext(nc) as tc, ExitStack() as ctx:
        persist = ctx.enter_context(tc.tile_pool(name="persist", bufs=1))

        # ---------------- constants ----------------
        ident = persist.tile([128, 128], F, name="c_ident")
        nc.sync.dma_start(ident[:], ident_d[:])
        ident_bf = persist.tile([128, 128], BF, name="c_ident_bf")
        nc.vector.tensor_copy(ident_bf[:], ident[:])
        triu_f = persist.tile([128, 128], F, name="c_triu_f")
        nc.sync.dma_start(triu_f[:], triu_d[:])
        triu = persist.tile([128, 128], BF, name="c_triu")
        nc.vector.tensor_copy(triu[:], triu_f[:])
        if32 = persist.tile([128, 32], F, name="c_if32")
        nc.sync.dma_start(if32[:], if32_d[:])
        if128 = persist.tile([128, 1], F, name="c_if128")
        nc.sync.dma_start(if128[:], if128_d[:])
        ones_f = persist.tile([128, 1], F, name="c_ones_f")
        nc.vector.memset(ones_f[:], 1.0)
        ones_fr = persist.tile([1, 128], F, name="c_ones_fr")
        nc.vector.memset(ones_fr[:], 1.0)
        ones_col = persist.tile([128, 1], BF, name="c_ones_col")
        nc.vector.tensor_copy(ones_col[:], ones_f[:])
        ones_row = persist.tile([1, 128], R, name="c_ones_row")
        nc.vector.tensor_copy(ones_row[:], ones_fr[:])
        eps_t = persist.tile([128, 1], F, name="c_eps")
        nc.vector.memset(eps_t[:], EPS)
        sin_k = [persist.tile([128, 32], F, name=f"t_sink{st}") for st in range(2)]
        cos_k = [persist.tile([128, 32], F, name=f"t_cosk{st}") for st in range(2)]

        # long-lived pools (stack-bottom): attention latents + o_proj staging
        kpe_p = ctx.enter_context(tc.tile_pool(name="kpe_p", bufs=1))
        vp = ctx.enter_context(tc.tile_pool(name="vp", bufs=1))
        knall = ctx.enter_context(tc.tile_pool(name="knall", bufs=1))
        owp = ctx.enter_context(tc.tile_pool(name="owp", bufs=1))
        g2p = ctx.enter_context(tc.tile_pool(name="g2p", bufs=2))

        kpe_rep = kpe_p.tile([128, S], BF, name="kpe_rep")
        v_sb = vp.tile([128, S // 128, 512], BF, name="v_sb")
        kn_all = [knall.tile([128, S], BF, name=f"kn{h}") for h in range(HPC)]

        # k_pe rope tables (local positions); scratch lives in persist so the
        # pool stack has no boundary here (xt DMAs must not wait on this)
        posf_loc = persist.tile([128, 2], F, name="posf_loc")
        pos_i_loc = persist.tile([128, 2], I32, name="pos_i_loc")
        nc.sync.dma_start(
            pos_i_loc[:], pos_loc.ap().rearrange("(t p) -> p t", p=128)
        )
        nc.vector.tensor_copy(posf_loc[:], pos_i_loc[:])
        for st in range(2):
            nc.vector.tensor_scalar_mul(
                sin_k[st][:], if32[:], posf_loc[:, st : st + 1]
            )
            nc.vector.tensor_scalar(
                cos_k[st][:], sin_k[st][:], np.pi / 2.0, None, ALU.add
            )
            _emit_range_reduce(nc, persist, sin_k[st][:], 32)
            _emit_range_reduce(nc, persist, cos_k[st][:], 32)
            nc.scalar.activation(sin_k[st][:], sin_k[st][:], AF.Sin)
            nc.scalar.activation(cos_k[st][:], cos_k[st][:], AF.Sin)

        ctxd1 = ExitStack()
        ckvp = ctxd1.enter_context(tc.tile_pool(name="ckvp", bufs=1))

        # ---------------- phase 0: q_a then kv_a projections ----------------
        ctx0 = ExitStack()
        xtp = ctx0.enter_context(tc.tile_pool(name="xtp", bufs=1))
        wp0 = ctx0.enter_context(tc.tile_pool(name="wp0", bufs=3))
        p0 = ctx0.enter_context(tc.tile_pool(name="p0", bufs=2))

        # x shard: 4 chunked DMAs on two queues so compute starts early
        xt = xtp.tile([128, NKT, SL], BF, name="xt")
        for q4 in range(4):
            eng = nc.sync if q4 % 2 == 0 else nc.scalar
            eng.dma_start(xt[:, ds(8 * q4, 8), :], xt_d.ap()[:, ds(8 * q4, 8), :])

        # gpsimd is free until its first collective trigger -> 3 DMA queues
        DMA_ROT = [nc.sync, nc.scalar, nc.gpsimd]

        # --- kv_a (chunks of 288 cols) ---
        ctx0a = ExitStack()
        kv_ps = ctx0a.enter_context(tc.tile_pool(name="kv_ps", bufs=4, space="PSUM"))
        tr_ps = ctx0a.enter_context(tc.tile_pool(name="tr_ps", bufs=2, space="PSUM"))
        trk_ps = ctx0a.enter_context(tc.tile_pool(name="trk_ps", bufs=1, space="PSUM"))
        trsb = ctx0a.enter_context(tc.tile_pool(name="trsb", bufs=3))

        kvch = [[None] * 2 for _ in range(2)]
        for ch in range(2):
            for st in range(2):
                kvch[st][ch] = kv_ps.tile(
                    [128, 288], F, tag="kv_ps", name=f"kvps{st}_{ch}"
                )
        for ch in range(2):
            for ktg in range(NKT // 8):
                w = wp0.tile([128, 8, 288], BF, tag="kvw", name=f"kvw{ch}_{ktg}")
                DMA_ROT[(ch * 4 + ktg) % 3].dma_start(w[:], kvaw_d.ap()[ch * 4 + ktg])
                for kk in range(8):
                    kt = ktg * 8 + kk
                    for st in range(2):
                        nc.tensor.matmul(
                            kvch[st][ch][:], xt[:, kt, ts(st, 128)], w[:, kk],
                            start=(kt == 0), stop=(kt == NKT - 1),
                        )
        for st in range(2):
            acc0 = p0.tile([128, 1], F, tag="kvacc", name=f"kvacc0_{st}")
            acc1 = p0.tile([128, 1], F, tag="kvacc", name=f"kvacc1_{st}")
            scr = p0.tile([128, 288], F, tag="kvscr", name=f"kvscr{st}")
            nc.scalar.activation(scr[:], kvch[st][0][:], AF.Square, accum_out=acc0[:])
            nc.scalar.activation(
                scr[:, 0:224], kvch[st][1][:, 0:224], AF.Square, accum_out=acc1[:]
            )
            nc.vector.tensor_tensor(acc0[:], acc0[:], acc1[:], ALU.add)
            stdv = p0.tile([128, 1], F, tag="kvstd", name=f"kvstd{st}")
            nc.scalar.activation(stdv[:], acc0[:], AF.Sqrt, bias=eps_t[:], scale=1.0 / KR)
            rinv = p0.tile([128, 1], F, tag="kvrinv", name=f"kvrinv{st}")
            nc.vector.reciprocal(rinv[:], stdv[:])
            ckvn = p0.tile([128, KR], BF, tag="ckvn", name=f"ckvn{st}")
            nc.vector.tensor_scalar_mul(ckvn[:, 0:288], kvch[st][0][:], rinv[:])
            nc.vector.tensor_scalar_mul(ckvn[:, 288:512], kvch[st][1][:, 0:224], rinv[:])
            # rope k_pe: cols 512:576 of kv_a = chunk1 cols 224:288, deinterleaved
            pe = kvch[st][1][:, 224:288].rearrange("p (d two) -> p two d", two=2)
            y1, y2 = pe[:, 0], pe[:, 1]
            kr_t = p0.tile([128, DR], F, tag="kr", name=f"kr{st}")
            t1 = p0.tile([128, 32], F, tag="krt1", name=f"krt1_{st}")
            t2 = p0.tile([128, 32], F, tag="krt2", name=f"krt2_{st}")
            nc.vector.tensor_tensor(t1[:], y1, cos_k[st][:], ALU.mult)
            nc.vector.tensor_tensor(t2[:], y2, sin_k[st][:], ALU.mult)
            nc.vector.tensor_tensor(kr_t[:, 0:32], t1[:], t2[:], ALU.subtract)
            nc.vector.tensor_tensor(t1[:], y2, cos_k[st][:], ALU.mult)
            nc.vector.tensor_tensor(t2[:], y1, sin_k[st][:], ALU.mult)
            nc.vector.tensor_tensor(kr_t[:, 32:64], t1[:], t2[:], ALU.add)
            for rt in range(KR // 128):
                tp = tr_ps.tile([128, 128], BF, tag="tr", name=f"kvtr{st}_{rt}")
                nc.tensor.transpose(tp[:], ckvn[:, ts(rt, 128)], ident_bf[:])
                sb_t = trsb.tile([128, 128], BF, tag="trsb", name=f"kvtrs{st}_{rt}")
                nc.any.tensor_copy(sb_t[:], tp[:])
                nc.sync.dma_start(g1kv_src.ap()[ts(rt, 128), ts(st, 128)], sb_t[:])
            tpk = trk_ps.tile([64, 128], F, tag="trk", name=f"kvtrk{st}")
            nc.tensor.transpose(tpk[:], kr_t[:], ident[:])
            sb_k = trsb.tile([64, 128], BF, tag="trsbk", name=f"kvtrks{st}")
            nc.any.tensor_copy(sb_k[:], tpk[:])
            nc.sync.dma_start(g1kv_src.ap()[KR : KR + DR, ts(st, 128)], sb_k[:])
        ctx0a.close()

        nc.gpsimd.collective_compute(
            "AllGather", ALU.bypass,
            ins=[g1kv_src.ap().opt()], outs=[g1kv.ap().opt()], replica_groups=RG,
        )

        # decompression inputs: issue on gpsimd right after the AG_kv trigger
        # so the transfers run during q_b compute
        for half in range(2):
            nc.gpsimd.dma_start(
                kpe_rep[ds(64 * half, 64), :].rearrange("p (c s) -> p c s", c=NCORES),
                g1kv.ap()[:, KR : KR + DR, :].rearrange("c p s -> p c s"),
            )
        ckv_t = []
        for r in range(NKR):
            t = ckvp.tile([128, S], BF, name=f"ckv{r}")
            nc.gpsimd.dma_start(
                t[:].rearrange("p (c s) -> p c s", c=NCORES),
                g1kv.ap()[:, ts(r, 128), :].rearrange("c p s -> p c s"),
            )
            ckv_t.append(t)
        kvbv, kvbn = [], []
        for r in range(NKR):
            tv = ckvp.tile([128, 512], BF, name=f"kvbv{r}")
            nc.gpsimd.dma_start(tv[:], kvbwT.ap()[ts(r, 128), 512:1024])
            kvbv.append(tv)
            tn = ckvp.tile([128, 512], BF, name=f"kvbn{r}")
            nc.gpsimd.dma_start(tn[:], kvbwT.ap()[ts(r, 128), 0:512])
            kvbn.append(tn)

        # --- q_a (chunks of 512 cols) ---
        ctx0b = ExitStack()
        qa_ps = ctx0b.enter_context(tc.tile_pool(name="qa_ps", bufs=6, space="PSUM"))
        tr2_ps = ctx0b.enter_context(tc.tile_pool(name="tr2_ps", bufs=2, space="PSUM"))
        tr2sb = ctx0b.enter_context(tc.tile_pool(name="tr2sb", bufs=3))
        qch = [[None] * 3 for _ in range(2)]
        for ch in range(3):
            for st in range(2):
                qch[st][ch] = qa_ps.tile(
                    [128, 512], F, tag="qa_ps", name=f"qaps{st}_{ch}"
                )
        for ch in range(3):
            for ktg in range(NKT // 8):
                w = wp0.tile([128, 8, 512], BF, tag="qaw", name=f"qaw{ch}_{ktg}")
                idx = ch * 4 + ktg
                (nc.sync if idx % 2 == 0 else nc.scalar).dma_start(
                    w[:], qaw_d.ap()[idx]
                )
                for kk in range(8):
                    kt = ktg * 8 + kk
                    for st in range(2):
                        nc.tensor.matmul(
                            qch[st][ch][:], xt[:, kt, ts(st, 128)], w[:, kk],
                            start=(kt == 0), stop=(kt == NKT - 1),
                        )
        qn_t = []
        for st in range(2):
            accs = []
            scr = p0.tile([128, 512], F, tag="qascr", name=f"qascr{st}")
            for ch in range(3):
                a = p0.tile([128, 1], F, tag="qaacc", name=f"qaacc{st}_{ch}")
                nc.scalar.activation(scr[:], qch[st][ch][:], AF.Square, accum_out=a[:])
                accs.append(a)
            nc.vector.tensor_tensor(accs[0][:], accs[0][:], accs[1][:], ALU.add)
            nc.vector.tensor_tensor(accs[0][:], accs[0][:], accs[2][:], ALU.add)
            stdv = p0.tile([128, 1], F, tag="qastd", name=f"qastd{st}")
            nc.scalar.activation(stdv[:], accs[0][:], AF.Sqrt, bias=eps_t[:], scale=1.0 / QR)
            rinv = p0.tile([128, 1], F, tag="qarinv", name=f"qarinv{st}")
            nc.vector.reciprocal(rinv[:], stdv[:])
            qn = p0.tile([128, QR], BF, tag="qn", name=f"qn{st}", bufs=2)
            for ch in range(3):
                nc.vector.tensor_scalar_mul(qn[:, ts(ch, 512)], qch[st][ch][:], rinv[:])
            qn_t.append(qn)
        # rt-major transposed writes: half A (rt 0-5) completes early so its
        # AllGather pipelines ahead of half B
        for half, g1q_src in ((0, g1q_srcA), (1, g1q_srcB)):
            for rt6 in range(6):
                rt = 6 * half + rt6
                for st in range(2):
                    tp = tr2_ps.tile([128, 128], BF, tag="tr2", name=f"qtr{st}_{rt}")
                    nc.tensor.transpose(tp[:], qn_t[st][:, ts(rt, 128)], ident_bf[:])
                    sb_t = tr2sb.tile([128, 128], BF, tag="tr2sb", name=f"qtrs{st}_{rt}")
                    nc.any.tensor_copy(sb_t[:], tp[:])
                    nc.sync.dma_start(
                        g1q_src.ap()[ts(rt6, 128), ts(st, 128)], sb_t[:]
                    )
            nc.gpsimd.collective_compute(
                "AllGather", ALU.bypass,
                ins=[(g1q_srcA if half == 0 else g1q_srcB).ap().opt()],
                outs=[(g1qA if half == 0 else g1qB).ap().opt()],
                replica_groups=RG,
            )
        ctx0b.close()

        ctx0.close()

        # ---------------- decompression (kn for h=0 first, then v, rest) ----
        ctxd2 = ExitStack()
        dec_ps = ctxd2.enter_context(tc.tile_pool(name="dec_ps", bufs=4, space="PSUM"))

        def emit_kn(h):
            for sb in range(4):
                ps = dec_ps.tile([128, 512], F, tag="dec", name=f"knps{h}_{sb}")
                for r in range(NKR):
                    nc.tensor.matmul(
                        ps[:], kvbn[r][:, ts(h, 128)], ckv_t[r][:, ts(sb, 512)],
                        start=(r == 0), stop=(r == NKR - 1),
                    )
                nc.any.tensor_copy(kn_all[h][:, ts(sb, 512)], ps[:])

        emit_kn(0)
        for st in range(S // 128):
            ps = dec_ps.tile([128, 512], F, tag="dec", name=f"vps{st}")
            for r in range(NKR):
                nc.tensor.matmul(
                    ps[:], ckv_t[r][:, ts(st, 128)], kvbv[r][:],
                    start=(r == 0), stop=(r == NKR - 1),
                )
            nc.any.tensor_copy(v_sb[:, st, :], ps[:])
        for h in range(1, HPC):
            emit_kn(h)
        ctxd2.close()
        ctxd1.close()

        # q_b outputs (live q_b -> attention); decompression inputs
        ctx_q = ExitStack()
        attbq = ctx_q.enter_context(tc.tile_pool(name="attbq", bufs=1))
        qnope = [attbq.tile([128, S], BF, name=f"qnope{h}") for h in range(HPC)]
        # rope q per head, duplicated into both 64-partition halves so rope
        # score matmuls for adjacent key-tiles can row-pack in the PE array
        qfpe = [attbq.tile([128, S], BF, name=f"qfpe{h}") for h in range(HPC)]

        # ---------------- phase 2: q_b projection (+ q rope) ----------------
        ctx2 = ExitStack()
        sinp = ctx2.enter_context(tc.tile_pool(name="sinp", bufs=1))
        tabq = ctx2.enter_context(tc.tile_pool(name="tabq", bufs=1))
        qrp = ctx2.enter_context(tc.tile_pool(name="qrp", bufs=12))
        qbwp = ctx2.enter_context(tc.tile_pool(name="qbwp", bufs=4))
        ropep = ctx2.enter_context(tc.tile_pool(name="ropep", bufs=2))
        qb_ps = ctx2.enter_context(tc.tile_pool(name="qb_ps", bufs=8, space="PSUM"))

        # sin/cos tables for q rope
        sin_q = sinp.tile([128, S], F, name="t_sinq")
        cos_q = sinp.tile([128, S], F, name="t_cosq")
        posf_row = tabq.tile([1, S], R, name="posf_row")
        pos_i_row = tabq.tile([1, S], I32, name="pos_i_row")
        nc.sync.dma_start(pos_i_row[:], pos_all[:])
        nc.vector.tensor_copy(posf_row[:], pos_i_row[:])
        for cchunk in range(4):
            cs = ts(cchunk, 512)
            bc = qb_ps.tile([128, 512], F, tag="qb_ps", name=f"tabbc{cchunk}")
            nc.tensor.matmul(
                bc[:], ones_row[:], posf_row[:, cs], start=True, stop=True,
            )
            nc.vector.tensor_scalar_mul(sin_q[:, cs], bc[:], if128[:])
            nc.vector.tensor_scalar(cos_q[:, cs], sin_q[:, cs], np.pi / 2.0, None, ALU.add)
            _emit_range_reduce(nc, tabq, sin_q[:, cs], 512)
            _emit_range_reduce(nc, tabq, cos_q[:, cs], 512)
            nc.scalar.activation(sin_q[:, cs], sin_q[:, cs], AF.Sin)
            nc.scalar.activation(cos_q[:, cs], cos_q[:, cs], AF.Sin)

        for sb in range(4):
            qr_tiles = []
            for r in range(NR):
                t = qrp.tile([128, 512], BF, tag="qr", name=f"qr{sb}_{r}")
                g1q_h = g1qA if r < 6 else g1qB
                nc.sync.dma_start(
                    t[:].rearrange("p (c s) -> p c s", c=2),
                    g1q_h.ap()[2 * sb : 2 * sb + 2, ts(r % 6, 128), :]
                    .rearrange("c p s -> p c s"),
                )
                qr_tiles.append(t)
            psj = [
                qb_ps.tile([128, 512], F, tag="qb_ps", name=f"qbps{sb}_{j}")
                for j in range(8)
            ]
            for r in range(NR):
                w = qbwp.tile([128, 1024], BF, tag="qbw", name=f"qbw{sb}_{r}")
                nc.scalar.dma_start(w[:], qbwT.ap()[ts(r, 128), :])
                for j in range(8):
                    nc.tensor.matmul(
                        psj[j][:], w[:, ts(j, 128)], qr_tiles[r][:],
                        start=(r == 0), stop=(r == NR - 1),
                    )
            for h in range(HPC):
                nc.any.tensor_copy(qnope[h][:, ts(sb, 512)], psj[h][:])
            for p in range(2):
                t1 = ropep.tile([128, 512], F, tag="rope1", name=f"rp1_{sb}_{p}")
                t2 = ropep.tile([128, 512], F, tag="rope2", name=f"rp2_{sb}_{p}")
                nc.vector.tensor_tensor(
                    t1[:], psj[4 + p][:], cos_q[:, ts(sb, 512)], ALU.mult
                )
                nc.vector.tensor_tensor(
                    t2[:], psj[6 + p][:], sin_q[:, ts(sb, 512)], ALU.mult
                )
                for hh in range(2):
                    sl = ds(64 * hh, 64)
                    for dh in range(2):
                        nc.vector.tensor_tensor(
                            qfpe[2 * p + hh][ds(64 * dh, 64), ts(sb, 512)],
                            t1[sl, :], t2[sl, :], ALU.add,
                        )
        ctx2.close()

        # prefetch o_proj weights on the scalar queue (3-slot rotation)
        ow_t = []
        for g in range(HPC):
            t = owp.tile([128, NCORES, MCOLS], BF, tag="ow", bufs=3, name=f"ow{g}")
            nc.sync.dma_start(t[:], ow_d.ap()[g])
            ow_t.append(t)

        # ---------------- phase 4: attention ----------------
        ctx3 = ExitStack()
        probp = ctx3.enter_context(tc.tile_pool(name="probp", bufs=6))
        dsump = ctx3.enter_context(tc.tile_pool(name="dsump", bufs=4))
        attup = ctx3.enter_context(tc.tile_pool(name="attup", bufs=2))
        attp = ctx3.enter_context(tc.tile_pool(name="attp", bufs=2))
        invp = ctx3.enter_context(tc.tile_pool(name="invp", bufs=2))
        denp = ctx3.enter_context(tc.tile_pool(name="denp", bufs=2))
        sc_ps = ctx3.enter_context(tc.tile_pool(name="sc_ps", bufs=5, space="PSUM"))
        av_ps = ctx3.enter_context(tc.tile_pool(name="av_ps", bufs=2, space="PSUM"))
        den_ps = ctx3.enter_context(tc.tile_pool(name="den_ps", bufs=1, space="PSUM"))

        PIPE = 3  # key-tiles of score-lookahead ahead of exp/av/den consumers
        pending = []  # deferred finalize/trigger emissions

        def flush_pending():
            while pending:
                pending.pop(0)()

        gts = {}

        for h in range(HPC):
            kn = kn_all[h]
            pe_rhs = qfpe[h]
            pe_lhs = kpe_rep
            for qb in range(4):
                nkt = 4 * (qb + 1)
                avt = av_ps.tile([128, 512], F, tag="av", name=f"av{h}_{qb}")
                dent = den_ps.tile([1, 512], F, tag="den", name=f"den{h}_{qb}")
                sct = [None] * nkt
                prob = [None] * nkt
                dsum = [None] * (qb + 1)

                def emit_nope(kt, h=h, qb=qb, kn=kn, sct=sct):
                    trim = max(0, 128 * (kt - 4 * qb))
                    qsl = ds(512 * qb + trim, 512 - trim)
                    sct[kt] = sc_ps.tile([128, 512], F, tag="sc", name=f"sc{h}{qb}_{kt}")
                    nc.tensor.matmul(
                        sct[kt][:, trim:512], kn[:, ts(kt, 128)], qnope[h][:, qsl],
                        start=True, stop=False,
                    )

                def emit_rope(kt, qb=qb, pe_rhs=pe_rhs, pe_lhs=pe_lhs, sct=sct):
                    # kt parity picks the PE row half -> adjacent pairs pack
                    trim = max(0, 128 * (kt - 4 * qb))
                    qsl = ds(512 * qb + trim, 512 - trim)
                    half = ds(64 * (kt % 2), 64)
                    nc.tensor.matmul(
                        sct[kt][:, trim:512], pe_lhs[half, ts(kt, 128)],
                        pe_rhs[half, qsl],
                        start=False, stop=True,
                    )

                def emit_post(kt, h=h, qb=qb, sct=sct, prob=prob, dsum=dsum):
                    trim = max(0, 128 * (kt - 4 * qb))
                    prob[kt] = probp.tile(
                        [128, 512], BF, tag="prob", name=f"pr{h}{qb}_{kt}"
                    )
                    nc.scalar.activation(
                        prob[kt][:, trim:512], sct[kt][:, trim:512], AF.Exp
                    )
                    if kt >= 4 * qb:
                        nc.vector.tensor_tensor(
                            prob[kt][:, trim : trim + 128],
                            prob[kt][:, trim : trim + 128],
                            triu[:],
                            ALU.mult,
                        )
                        # diagonal group: columns below each tile's trim get no
                        # contribution (causal), so sliced adds need no masking
                        if kt == 4 * qb:
                            dsum[qb] = dsump.tile(
                                [128, 512], BF, tag="dsum", name=f"dsd{h}{qb}"
                            )
                            nc.vector.tensor_copy(dsum[qb][:], prob[kt][:])
                        else:
                            nc.vector.tensor_tensor(
                                dsum[qb][:, trim:512], dsum[qb][:, trim:512],
                                prob[kt][:, trim:512], ALU.add,
                            )
                    else:
                        # group partial prob sums (full tiles only) so the
                        # denominator needs one matmul per 4 key-tiles
                        g4 = kt // 4
                        if kt % 4 == 1:
                            dsum[g4] = dsump.tile(
                                [128, 512], BF, tag="dsum", name=f"ds{h}{qb}_{g4}"
                            )
                            nc.vector.tensor_tensor(
                                dsum[g4][:], prob[kt - 1][:], prob[kt][:], ALU.add
                            )
                        elif kt % 4 >= 2:
                            nc.vector.tensor_tensor(
                                dsum[g4][:], dsum[g4][:], prob[kt][:], ALU.add
                            )

                def emit_scores(kt):
                    emit_nope(kt)
                    if kt % 2 == 1:
                        emit_rope(kt - 1)
                        emit_rope(kt)
                        emit_post(kt - 1)
                        emit_post(kt)

                def emit_av(kt, h=h, qb=qb, avt=avt, dent=dent, prob=prob,
                            dsum=dsum, nkt=nkt):
                    trim = max(0, 128 * (kt - 4 * qb))
                    nc.tensor.matmul(
                        avt[:, trim:512], v_sb[:, kt, ts(h, 128)],
                        prob[kt][:, trim:512],
                        start=(kt == 0), stop=(kt == nkt - 1),
                    )
                    if kt < 4 * qb:
                        if kt % 4 == 3:
                            nc.tensor.matmul(
                                dent[:], ones_col[:], dsum[kt // 4][:],
                                start=(kt == 3), stop=False,
                            )
                    elif kt == nkt - 1:
                        nc.tensor.matmul(
                            dent[:], ones_col[:], dsum[qb][:],
                            start=(qb == 0), stop=True,
                        )

                for kt in range(min(PIPE, nkt)):
                    emit_scores(kt)
                flush_pending()
                for kt in range(nkt):
                    if kt + PIPE < nkt:
                        emit_scores(kt + PIPE)
                    emit_av(kt)

                den_sb = denp.tile([1, 512], R, tag="densb", name=f"densb{h}_{qb}")
                nc.vector.tensor_copy(den_sb[:], dent[:])
                att_u = attup.tile([128, 512], F, tag="attu", name=f"attu{h}_{qb}")
                nc.vector.tensor_copy(att_u[:], avt[:])

                def finalize(h=h, qb=qb, den_sb=den_sb, att_u=att_u):
                    bc = sc_ps.tile([128, 512], F, tag="sc", name=f"bc{h}_{qb}")
                    nc.tensor.matmul(bc[:], ones_row[:], den_sb[:], start=True, stop=True)
                    inv = invp.tile([128, 512], F, tag="inv", name=f"inv{h}_{qb}")
                    nc.vector.reciprocal_approx_fast(inv[:], bc[:])
                    att = attp.tile([128, 512], BF, tag="att", name=f"att{h}_{qb}")
                    nc.vector.tensor_tensor(att[:], att_u[:], inv[:], ALU.mult)
                    nc.gpsimd.dma_start(g2_src[h].ap()[:, ts(qb, 512)], att[:])

                pending.append(finalize)

            def trigger(h=h):
                nc.gpsimd.collective_compute(
                    "AllGather", ALU.bypass,
                    ins=[g2_src[h].ap().opt()], outs=[g2[h].ap().opt()],
                    replica_groups=RG,
                )
                for sblk in range(4):
                    gt = g2p.tile(
                        [128, NCORES, 512], BF, tag="g2t", name=f"g2t{h}_{sblk}"
                    )
                    nc.sync.dma_start(
                        gt[:],
                        g2[h].ap()[:, :, ts(sblk, 512)].rearrange("c p s -> p c s"),
                    )
                    gts[(h, sblk)] = gt

            pending.append(trigger)
        flush_pending()
        ctx3.close()
        ctx_q.close()

        # ---------------- phase 5: o_proj, transposed (out_T[m, s]) ----------------
        ctx5 = ExitStack()
        osb = ctx5.enter_context(tc.tile_pool(name="osb", bufs=1))
        o_ps = ctx5.enter_context(tc.tile_pool(name="o_ps", bufs=4, space="PSUM"))

        oacc = [
            osb.tile([128, 512], F, tag=f"oacc{i}", name=f"oacc{i}")
            for i in range(16)
        ]
        for g in range(HPC):
            for sblk in range(4):
                gt = gts[(g, sblk)]
                for mt in range(MCOLS // 128):
                    ps = o_ps.tile([128, 512], F, tag="o", name=f"ops{g}_{sblk}_{mt}")
                    for r8 in range(NCORES):
                        nc.tensor.matmul(
                            ps[:], ow_t[g][:, r8, ts(mt, 128)], gt[:, r8, :],
                            start=(r8 == 0), stop=(r8 == NCORES - 1),
                        )
                    acc = oacc[sblk * 4 + mt]
                    if g == 0:
                        nc.any.tensor_copy(acc[:], ps[:])
                    else:
                        nc.vector.tensor_tensor(acc[:], acc[:], ps[:], ALU.add)
                    if g == HPC - 1:
                        nc.gpsimd.dma_start(
                            out_d.ap()[ts(mt, 128), ts(sblk, 512)], acc[:]
                        )
        ctx5.close()

    nc.compile()
    return nc


def _get_nc():
    global _CACHED_NC
    if _CACHED_NC is None:
        _CACHED_NC = _build_program()
    return _CACHED_NC


def _prep_inputs(hidden_states, position_ids, q_a_w, q_a_ln_w, q_b_w, kv_a_w,
                 kv_a_ln_w, kv_b_w, o_w):
    hidden_states = np.asarray(hidden_states, dtype=np.float32)
    position_ids = np.asarray(position_ids, dtype=np.int32)
    q_a_w = np.asarray(q_a_w, dtype=np.float32)
    q_a_ln_w = np.asarray(q_a_ln_w, dtype=np.float32)
    q_b_w = np.asarray(q_b_w, dtype=np.float32)
    kv_a_w = np.asarray(kv_a_w, dtype=np.float32)
    kv_a_ln_w = np.asarray(kv_a_ln_w, dtype=np.float32)
    kv_b_w = np.asarray(kv_b_w, dtype=np.float32)
    o_w = np.asarray(o_w, dtype=np.float32)
    bf = ml_dtypes.bfloat16

    xT_full = np.ascontiguousarray(hidden_states[0].T)          # (H, S)
    qawT = np.ascontiguousarray(q_a_w.T)                         # (H, QR)
    kvawT = np.ascontiguousarray(kv_a_w.T)                       # (H, KR+DR)
    ident = np.eye(128, dtype=np.float32)
    triu = np.triu(np.ones((128, 128), dtype=np.float32))        # keep k <= q
    inv_freq = (1.0 / (THETA ** (np.arange(0, DR, 2) / DR))).astype(np.float32)
    if32 = np.tile(inv_freq[None, :], (128, 1)).astype(np.float32)
    if128 = np.tile(inv_freq, 4)[:, None].astype(np.float32)
    owT_full = np.ascontiguousarray(o_w.T)                       # (N*DV, H)

    # DMA-contiguous weight layouts for the q_a / kv_a streams
    qaw_l = np.empty((12, 128, 8, 512), dtype=bf)
    for ch in range(3):
        for ktg in range(4):
            blk = qawT[ktg * 1024 : (ktg + 1) * 1024, ch * 512 : (ch + 1) * 512]
            qaw_l[ch * 4 + ktg] = blk.reshape(8, 128, 512).transpose(1, 0, 2)
    kvaw_l = np.empty((8, 128, 8, 288), dtype=bf)
    for ch in range(2):
        for ktg in range(4):
            blk = kvawT[ktg * 1024 : (ktg + 1) * 1024, ch * 288 : (ch + 1) * 288]
            kvaw_l[ch * 4 + ktg] = blk.reshape(8, 128, 288).transpose(1, 0, 2)

    in_maps = []
    for c in range(NCORES):
        heads = slice(HPC * c, HPC * (c + 1))
        qb = q_b_w.reshape(N_HEADS, QD, QR)[heads]               # (4, 192, QR)
        nope = qb[:, :DN, :].reshape(HPC * DN, QR)
        pe = qb[:, DN:, :]
        pe_d = np.concatenate([pe[:, 0::2, :], pe[:, 1::2, :]], axis=1)  # (4,64,QR)
        pe_r = np.concatenate([-pe_d[:, 32:, :], pe_d[:, :32, :]], axis=1)
        cols = np.concatenate(
            [nope, pe_d.reshape(HPC * DR, QR), pe_r.reshape(HPC * DR, QR)], axis=0
        )                                                        # (1024, QR)
        qbwT_c = np.ascontiguousarray((cols * (SCALE * q_a_ln_w[None, :])).T)

        kvb = kv_b_w.reshape(N_HEADS, DN + DV, KR)[heads]
        kcols = np.concatenate(
            [kvb[:, :DN, :].reshape(HPC * DN, KR),
             kvb[:, DN:, :].reshape(HPC * DV, KR)],
            axis=0,
        )                                                        # (1024, KR)
        kvbwT_c = np.ascontiguousarray((kcols * kv_a_ln_w[None, :]).T)

        # o_w slice in [g][p][r8][m] layout
        ow_c = owT_full[:, MCOLS * c : MCOLS * (c + 1)]          # (4096, 512)
        ow_l = np.ascontiguousarray(
            ow_c.reshape(NCORES, HPC, 128, MCOLS).transpose(1, 2, 0, 3)
        ).astype(bf)                                             # (4, 128, 8, 512)

        xt_shard = xT_full[:, SL * c : SL * (c + 1)]             # (4096, 256)
        xt_l = np.ascontiguousarray(
            xt_shard.reshape(32, 128, SL).transpose(1, 0, 2)
        ).astype(bf)                                             # (128, 32, 256)

        in_maps.append(
            {
                "xt_d": xt_l,
                "qaw_d": qaw_l,
                "kvaw_d": kvaw_l,
                "qbwT": qbwT_c.astype(bf),
                "kvbwT": kvbwT_c.astype(bf),
                "ow_d": ow_l,
                "pos_all": np.ascontiguousarray(position_ids.reshape(1, S)),
                "pos_loc": np.ascontiguousarray(
                    position_ids.reshape(-1)[SL * c : SL * (c + 1)]
                ),
                "ident": ident,
                "triu": triu,
                "if32": if32,
                "if128": if128,
            }
        )
    return in_maps


def kernel(**inputs):
    global LAST_RESULT
    nc = _get_nc()
    in_maps = _prep_inputs(**inputs)
    res = run_bass_kernel_spmd(nc, in_maps, list(range(NCORES)))
    LAST_RESULT = res
    out = np.concatenate([res.results[c]["out"].T for c in range(NCORES)], axis=1)
    return out[None].astype(np.float32)


# revision 42
# speedup vs baseline: 1.0340x; 1.0340x over previous
"""DeepSeekV3 MLA attention prefill kernel for 8 Trainium2 NeuronCores.

Sharding: sequence-parallel for the low-rank input projections (q_a / kv_a),
AllGather of the shared latents, tensor-parallel over heads (4 heads/core) for
q_b / kv_b decompression and attention, AllGather of attention outputs, and
output-column-parallel o_proj (each core produces a disjoint 512-column slice
of the output, concatenated on host).

Schedule notes (final):
- kv_a runs first so AG_kv is the first collective (absorbs the ncfw
  pickup + init-barrier latency); its latents DMA in during q_a
- AG_q is split into two pipelined halves (rt-major transposed writes) so
  q_b can start contracting on the first half while the second transfers
- kv_b decompression emitted between q_a and q_b, filling the AG_q window
- input/weight DMAs spread across sync/scalar/gpsimd issue queues with
  host-side DMA-contiguous weight layouts
- attention software-pipelined 3 key-tiles ahead of the exp chain; softmax
  finalize (den broadcast / fast-approx reciprocal / g2 write) deferred
  behind the next block's matmuls so the PE never drains at boundaries
- o_proj: g2 reads prefetched per head-group as its AllGather lands,
  deep PSUM buffering, SBUF accumulation across head-groups
"""

import sys

sys.path.insert(0, "/opt/trn_rl_repo")

import numpy as np
import ml_dtypes

import concourse.bass as bass  # noqa: F401
import concourse.mybir as mybir
from concourse import bacc
from concourse.bass import ds, ts
from concourse.tile import TileContext
from concourse.bass_utils import run_bass_kernel_spmd
from contextlib import ExitStack

F = mybir.dt.float32
BF = mybir.dt.bfloat16
R = mybir.dt.float32r
I32 = mybir.dt.int32
F8 = mybir.dt.float8e4
AF = mybir.ActivationFunctionType
ALU = mybir.AluOpType

NCORES = 8
B, S, H = 1, 2048, 4096
N_HEADS = 32
HPC = N_HEADS // NCORES          # heads per core = 4
SL = S // NCORES                 # sequence rows per core = 256
QR, KR = 1536, 512
DR, DN, DV = 64, 128, 128
QD = DN + DR                     # 192
SCALE = QD ** -0.5
EPS = 1e-6
THETA = 10000.0
TWO_PI = float(2.0 * np.pi)
MCOLS = H // NCORES              # output columns per core = 512

LAST_RESULT = None               # test harness reads exec_time_ns from here
_CACHED_NC = None
_UID = [0]


def _uid():
    _UID[0] += 1
    return _UID[0]


def _emit_range_reduce(nc, pool, t_ap, width):
    """In-place wrap t_ap (f32, [128, width]) to [-pi, pi]. f32->i32 copy
    rounds to nearest (verified on HW)."""
    tn = pool.tile([128, width], F, tag=f"rr_f_{width}", name=f"rrf{_uid()}")
    ti = pool.tile([128, width], I32, tag=f"rr_i_{width}", name=f"rri{_uid()}")
    nc.vector.tensor_scalar_mul(tn[:], t_ap, 1.0 / TWO_PI)
    nc.vector.tensor_copy(ti[:], tn[:])
    nc.vector.tensor_copy(tn[:], ti[:])
    nc.vector.tensor_scalar_mul(tn[:], tn[:], -TWO_PI)
    nc.vector.tensor_tensor(t_ap, t_ap, tn[:], ALU.add)


def _build_program():
    nc = bacc.Bacc(None, target_bir_lowering=False, num_devices=NCORES)

    # ---------------- DRAM declarations ----------------
    xt_d = nc.dram_tensor("xt_d", [128, 32, SL], BF, kind="ExternalInput")
    qaw_d = nc.dram_tensor("qaw_d", [12, 128, 8, 512], BF, kind="ExternalInput")
    kvaw_d = nc.dram_tensor("kvaw_d", [8, 128, 8, 288], BF, kind="ExternalInput")
    qbwT = nc.dram_tensor("qbwT", [QR, 1024], BF, kind="ExternalInput")
    kvbwT = nc.dram_tensor("kvbwT", [KR, 1024], BF, kind="ExternalInput")
    ow_d = nc.dram_tensor("ow_d", [HPC, 128, NCORES, MCOLS], BF, kind="ExternalInput")
    ow8_d = nc.dram_tensor("ow8_d", [128, NCORES, MCOLS], F8, kind="ExternalInput")
    pos_all = nc.dram_tensor("pos_all", [1, S], I32, kind="ExternalInput")
    pos_loc = nc.dram_tensor("pos_loc", [SL], I32, kind="ExternalInput")
    ident_d = nc.dram_tensor("ident", [128, 128], F, kind="ExternalInput")
    triu_d = nc.dram_tensor("triu", [128, 128], F, kind="ExternalInput")
    if32_d = nc.dram_tensor("if32", [128, 32], F, kind="ExternalInput")
    if128_d = nc.dram_tensor("if128", [128, 1], F, kind="ExternalInput")
    out_d = nc.dram_tensor("out", [MCOLS, S], F, kind="ExternalOutput")

    g1kv_src = nc.dram_tensor("g1kv_src", [KR + DR, SL], BF)
    g1kv = nc.dram_tensor("g1kv", [NCORES, KR + DR, SL], BF, addr_space="Shared")
    QH = QR // 2
    g1q_srcA = nc.dram_tensor("g1q_srcA", [QH, SL], BF)
    g1qA = nc.dram_tensor("g1qA", [NCORES, QH, SL], BF, addr_space="Shared")
    g1q_srcB = nc.dram_tensor("g1q_srcB", [QH, SL], BF)
    g1qB = nc.dram_tensor("g1qB", [NCORES, QH, SL], BF, addr_space="Shared")
    warm_src = nc.dram_tensor("warm_src", [1, 128], BF)
    warm_dst = nc.dram_tensor("warm_dst", [NCORES, 1, 128], BF, addr_space="Shared")
    G2DT = [F8, BF, BF, BF]
    g2_src = [nc.dram_tensor(f"g2src{h}", [DV, S], G2DT[h]) for h in range(HPC)]
    g2 = [
        nc.dram_tensor(f"g2_{h}", [NCORES, DV, S], G2DT[h], addr_space="Shared")
        for h in range(HPC)
    ]
    RG = [list(range(NCORES))]
    NKT = H // 128  # 32 k-tiles over the model dim
    NR = QR // 128  # 12 k-tiles over q_lora_rank
    NKR = KR // 128  # 4 k-tiles over kv_lora_rank

    with TileContext(nc) as tc, ExitStack() as ctx:
        persist = ctx.enter_context(tc.tile_pool(name="persist", bufs=1))

        # ---------------- constants ----------------
        ident = persist.tile([128, 128], F, name="c_ident")
        nc.gpsimd.dma_start(ident[:], ident_d[:])
        triu_f = persist.tile([128, 128], F, name="c_triu_f")
        nc.gpsimd.dma_start(triu_f[:], triu_d[:])
        triu = persist.tile([128, 128], BF, name="c_triu")
        nc.vector.tensor_copy(triu[:], triu_f[:])
        if32 = persist.tile([128, 32], F, name="c_if32")
        nc.gpsimd.dma_start(if32[:], if32_d[:])
        if128 = persist.tile([128, 1], F, name="c_if128")
        nc.gpsimd.dma_start(if128[:], if128_d[:])
        ones_f = persist.tile([128, 1], F, name="c_ones_f")
        nc.vector.memset(ones_f[:], 1.0)
        ones_fr = persist.tile([1, 128], F, name="c_ones_fr")
        nc.vector.memset(ones_fr[:], 1.0)
        ones_col = persist.tile([128, 1], BF, name="c_ones_col")
        nc.vector.tensor_copy(ones_col[:], ones_f[:])
        ones_row = persist.tile([1, 128], R, name="c_ones_row")
        nc.vector.tensor_copy(ones_row[:], ones_fr[:])
        inv64_fr = persist.tile([1, 128], F, name="c_inv64_fr")
        nc.vector.memset(inv64_fr[:], 1.0 / 64.0)
        inv64_row = persist.tile([1, 128], R, name="c_inv64_row")
        nc.vector.tensor_copy(inv64_row[:], inv64_fr[:])
        eps_t = persist.tile([128, 1], F, name="c_eps")
        nc.vector.memset(eps_t[:], EPS)
        sin_k = [persist.tile([128, 32], F, name=f"t_sink{st}") for st in range(2)]
        cos_k = [persist.tile([128, 32], F, name=f"t_cosk{st}") for st in range(2)]

        # long-lived pools (stack-bottom): attention latents + o_proj staging
        kpe_p = ctx.enter_context(tc.tile_pool(name="kpe_p", bufs=1))
        vp = ctx.enter_context(tc.tile_pool(name="vp", bufs=1))
        knall = ctx.enter_context(tc.tile_pool(name="knall", bufs=1))
        owp = ctx.enter_context(tc.tile_pool(name="owp", bufs=1))
        g2p = ctx.enter_context(tc.tile_pool(name="g2p", bufs=2))

        kpe_rep = kpe_p.tile([128, S], BF, name="kpe_rep")
        v_sb = vp.tile([128, S // 128, 512], BF, name="v_sb")
        kn_all = [knall.tile([128, S], BF, name=f"kn{h}") for h in range(HPC)]

        # k_pe rope tables (local positions); scratch lives in persist so the
        # pool stack has no boundary here (xt DMAs must not wait on this)
        posf_loc = persist.tile([128, 2], F, name="posf_loc")
        pos_i_loc = persist.tile([128, 2], I32, name="pos_i_loc")
        nc.gpsimd.dma_start(
            pos_i_loc[:], pos_loc.ap().rearrange("(t p) -> p t", p=128)
        )
        nc.vector.tensor_copy(posf_loc[:], pos_i_loc[:])
        for st in range(2):
            nc.vector.tensor_scalar_mul(
                sin_k[st][:], if32[:], posf_loc[:, st : st + 1]
            )
            nc.vector.tensor_scalar(
                cos_k[st][:], sin_k[st][:], np.pi / 2.0, None, ALU.add
            )
            _emit_range_reduce(nc, persist, sin_k[st][:], 32)
            _emit_range_reduce(nc, persist, cos_k[st][:], 32)
            nc.scalar.activation(sin_k[st][:], sin_k[st][:], AF.Sin)
            nc.scalar.activation(cos_k[st][:], cos_k[st][:], AF.Sin)

        ctxd1 = ExitStack()
        ckvp = ctxd1.enter_context(tc.tile_pool(name="ckvp", bufs=1))

        # ---------------- phase 0: q_a then kv_a projections ----------------
        ctx0 = ExitStack()
        xtp = ctx0.enter_context(tc.tile_pool(name="xtp", bufs=1))
        wp0 = ctx0.enter_context(tc.tile_pool(name="wp0", bufs=3))
        p0 = ctx0.enter_context(tc.tile_pool(name="p0", bufs=2))

        # x shard: 4 chunked DMAs on two queues so compute starts early
        xt = xtp.tile([128, NKT, SL], BF, name="xt")
        for q4 in range(4):
            eng = nc.sync if q4 % 2 == 0 else nc.scalar
            eng.dma_start(xt[:, ds(8 * q4, 8), :], xt_d.ap()[:, ds(8 * q4, 8), :])

        # gpsimd is free until its first collective trigger -> 3 DMA queues
        DMA_ROT = [nc.sync, nc.scalar, nc.gpsimd]

        # --- kv_a (chunks of 288 cols) ---
        ctx0a = ExitStack()
        kv_ps = ctx0a.enter_context(tc.tile_pool(name="kv_ps", bufs=4, space="PSUM"))
        tr_ps = ctx0a.enter_context(tc.tile_pool(name="tr_ps", bufs=2, space="PSUM"))
        trk_ps = ctx0a.enter_context(tc.tile_pool(name="trk_ps", bufs=1, space="PSUM"))
        trsb = ctx0a.enter_context(tc.tile_pool(name="trsb", bufs=3))

        kvch = [[None] * 2 for _ in range(2)]
        for ch in range(2):
            for st in range(2):
                kvch[st][ch] = kv_ps.tile(
                    [128, 288], F, tag="kv_ps", name=f"kvps{st}_{ch}"
                )
        for ch in range(2):
            for ktg in range(NKT // 8):
                w = wp0.tile([128, 8, 288], BF, tag="kvw", name=f"kvw{ch}_{ktg}")
                DMA_ROT[(ch * 4 + ktg) % 3].dma_start(w[:], kvaw_d.ap()[ch * 4 + ktg])
                for kk in range(8):
                    kt = ktg * 8 + kk
                    for st in range(2):
                        nc.tensor.matmul(
                            kvch[st][ch][:], xt[:, kt, ts(st, 128)], w[:, kk],
                            start=(kt == 0), stop=(kt == NKT - 1),
                        )
        for st in range(2):
            acc0 = p0.tile([128, 1], F, tag="kvacc", name=f"kvacc0_{st}")
            acc1 = p0.tile([128, 1], F, tag="kvacc", name=f"kvacc1_{st}")
            scr = p0.tile([128, 288], F, tag="kvscr", name=f"kvscr{st}")
            nc.scalar.activation(scr[:], kvch[st][0][:], AF.Square, accum_out=acc0[:])
            nc.scalar.activation(
                scr[:, 0:224], kvch[st][1][:, 0:224], AF.Square, accum_out=acc1[:]
            )
            nc.vector.tensor_tensor(acc0[:], acc0[:], acc1[:], ALU.add)
            stdv = p0.tile([128, 1], F, tag="kvstd", name=f"kvstd{st}")
            nc.scalar.activation(stdv[:], acc0[:], AF.Sqrt, bias=eps_t[:], scale=1.0 / KR)
            rinv = p0.tile([128, 1], F, tag="kvrinv", name=f"kvrinv{st}")
            nc.vector.reciprocal(rinv[:], stdv[:])
            ckvn = p0.tile([128, KR], F, tag="ckvn", name=f"ckvn{st}")
            nc.vector.tensor_scalar_mul(ckvn[:, 0:288], kvch[st][0][:], rinv[:])
            nc.vector.tensor_scalar_mul(ckvn[:, 288:512], kvch[st][1][:, 0:224], rinv[:])
            # rope k_pe: cols 512:576 of kv_a = chunk1 cols 224:288, deinterleaved
            pe = kvch[st][1][:, 224:288].rearrange("p (d two) -> p two d", two=2)
            y1, y2 = pe[:, 0], pe[:, 1]
            kr_t = p0.tile([128, DR], F, tag="kr", name=f"kr{st}")
            t1 = p0.tile([128, 32], F, tag="krt1", name=f"krt1_{st}")
            t2 = p0.tile([128, 32], F, tag="krt2", name=f"krt2_{st}")
            nc.vector.tensor_tensor(t1[:], y1, cos_k[st][:], ALU.mult)
            nc.vector.tensor_tensor(t2[:], y2, sin_k[st][:], ALU.mult)
            nc.vector.tensor_tensor(kr_t[:, 0:32], t1[:], t2[:], ALU.subtract)
            nc.vector.tensor_tensor(t1[:], y2, cos_k[st][:], ALU.mult)
            nc.vector.tensor_tensor(t2[:], y1, sin_k[st][:], ALU.mult)
            nc.vector.tensor_tensor(kr_t[:, 32:64], t1[:], t2[:], ALU.add)
            for rt in range(KR // 128):
                tp = tr_ps.tile([128, 128], F, tag="tr", name=f"kvtr{st}_{rt}")
                nc.tensor.transpose(tp[:], ckvn[:, ts(rt, 128)], ident[:])
                sb_t = trsb.tile([128, 128], BF, tag="trsb", name=f"kvtrs{st}_{rt}")
                nc.any.tensor_copy(sb_t[:], tp[:])
                nc.sync.dma_start(g1kv_src.ap()[ts(rt, 128), ts(st, 128)], sb_t[:])
            tpk = trk_ps.tile([64, 128], F, tag="trk", name=f"kvtrk{st}")
            nc.tensor.transpose(tpk[:], kr_t[:], ident[:])
            sb_k = trsb.tile([64, 128], BF, tag="trsbk", name=f"kvtrks{st}")
            nc.any.tensor_copy(sb_k[:], tpk[:])
            nc.sync.dma_start(g1kv_src.ap()[KR : KR + DR, ts(st, 128)], sb_k[:])
        ctx0a.close()

        nc.gpsimd.collective_compute(
            "AllGather", ALU.bypass,
            ins=[g1kv_src.ap().opt()], outs=[g1kv.ap().opt()], replica_groups=RG,
        )

        # decompression inputs: issue on gpsimd right after the AG_kv trigger
        # so the transfers run during q_b compute
        for half in range(2):
            nc.gpsimd.dma_start(
                kpe_rep[ds(64 * half, 64), :].rearrange("p (c s) -> p c s", c=NCORES),
                g1kv.ap()[:, KR : KR + DR, :].rearrange("c p s -> p c s"),
            )
        ckv_t = []
        for r in range(NKR):
            t = ckvp.tile([128, S], BF, name=f"ckv{r}")
            nc.gpsimd.dma_start(
                t[:].rearrange("p (c s) -> p c s", c=NCORES),
                g1kv.ap()[:, ts(r, 128), :].rearrange("c p s -> p c s"),
            )
            ckv_t.append(t)
        kvbv, kvbn = [], []
        for r in range(NKR):
            tv = ckvp.tile([128, 512], BF, name=f"kvbv{r}")
            nc.gpsimd.dma_start(tv[:], kvbwT.ap()[ts(r, 128), 512:1024])
            kvbv.append(tv)
            tn = ckvp.tile([128, 512], BF, name=f"kvbn{r}")
            nc.gpsimd.dma_start(tn[:], kvbwT.ap()[ts(r, 128), 0:512])
            kvbn.append(tn)

        # --- q_a (chunks of 512 cols) ---
        ctx0b = ExitStack()
        qa_ps = ctx0b.enter_context(tc.tile_pool(name="qa_ps", bufs=6, space="PSUM"))
        tr2_ps = ctx0b.enter_context(tc.tile_pool(name="tr2_ps", bufs=2, space="PSUM"))
        tr2sb = ctx0b.enter_context(tc.tile_pool(name="tr2sb", bufs=3))
        qch = [[None] * 3 for _ in range(2)]
        for ch in range(3):
            for st in range(2):
                qch[st][ch] = qa_ps.tile(
                    [128, 512], F, tag="qa_ps", name=f"qaps{st}_{ch}"
                )
        for ch in range(3):
            for ktg in range(NKT // 8):
                w = wp0.tile([128, 8, 512], BF, tag="qaw", name=f"qaw{ch}_{ktg}")
                idx = ch * 4 + ktg
                (nc.sync if idx % 2 == 0 else nc.scalar).dma_start(
                    w[:], qaw_d.ap()[idx]
                )
                for kk in range(8):
                    kt = ktg * 8 + kk
                    for st in range(2):
                        nc.tensor.matmul(
                            qch[st][ch][:], xt[:, kt, ts(st, 128)], w[:, kk],
                            start=(kt == 0), stop=(kt == NKT - 1),
                        )
        qn_t = []
        for st in range(2):
            accs = []
            scr = p0.tile([128, 512], F, tag="qascr", name=f"qascr{st}")
            for ch in range(3):
                a = p0.tile([128, 1], F, tag="qaacc", name=f"qaacc{st}_{ch}")
                nc.scalar.activation(scr[:], qch[st][ch][:], AF.Square, accum_out=a[:])
                accs.append(a)
            nc.vector.tensor_tensor(accs[0][:], accs[0][:], accs[1][:], ALU.add)
            nc.vector.tensor_tensor(accs[0][:], accs[0][:], accs[2][:], ALU.add)
            stdv = p0.tile([128, 1], F, tag="qastd", name=f"qastd{st}")
            nc.scalar.activation(stdv[:], accs[0][:], AF.Sqrt, bias=eps_t[:], scale=1.0 / QR)
            rinv = p0.tile([128, 1], F, tag="qarinv", name=f"qarinv{st}")
            nc.vector.reciprocal(rinv[:], stdv[:])
            qn = p0.tile([128, QR], F, tag="qn", name=f"qn{st}", bufs=2)
            for ch in range(3):
                nc.vector.tensor_scalar_mul(qn[:, ts(ch, 512)], qch[st][ch][:], rinv[:])
            qn_t.append(qn)
        # rt-major transposed writes: half A (rt 0-5) completes early so its
        # AllGather pipelines ahead of half B
        for half, g1q_src in ((0, g1q_srcA), (1, g1q_srcB)):
            for rt6 in range(6):
                rt = 6 * half + rt6
                for st in range(2):
                    tp = tr2_ps.tile([128, 128], F, tag="tr2", name=f"qtr{st}_{rt}")
                    nc.tensor.transpose(tp[:], qn_t[st][:, ts(rt, 128)], ident[:])
                    sb_t = tr2sb.tile([128, 128], BF, tag="tr2sb", name=f"qtrs{st}_{rt}")
                    nc.any.tensor_copy(sb_t[:], tp[:])
                    nc.sync.dma_start(
                        g1q_src.ap()[ts(rt6, 128), ts(st, 128)], sb_t[:]
                    )
            nc.gpsimd.collective_compute(
                "AllGather", ALU.bypass,
                ins=[(g1q_srcA if half == 0 else g1q_srcB).ap().opt()],
                outs=[(g1qA if half == 0 else g1qB).ap().opt()],
                replica_groups=RG,
            )
        ctx0b.close()

        ctx0.close()

        # ---------------- decompression (kn for h=0 first, then v, rest) ----
        ctxd2 = ExitStack()
        dec_ps = ctxd2.enter_context(tc.tile_pool(name="dec_ps", bufs=4, space="PSUM"))

        def emit_kn(h):
            for sb in range(4):
                ps = dec_ps.tile([128, 512], F, tag="dec", name=f"knps{h}_{sb}")
                for r in range(NKR):
                    nc.tensor.matmul(
                        ps[:], kvbn[r][:, ts(h, 128)], ckv_t[r][:, ts(sb, 512)],
                        start=(r == 0), stop=(r == NKR - 1),
                    )
                nc.any.tensor_copy(kn_all[h][:, ts(sb, 512)], ps[:])

        emit_kn(0)
        for st in range(S // 128):
            ps = dec_ps.tile([128, 512], F, tag="dec", name=f"vps{st}")
            for r in range(NKR):
                nc.tensor.matmul(
                    ps[:], ckv_t[r][:, ts(st, 128)], kvbv[r][:],
                    start=(r == 0), stop=(r == NKR - 1),
                )
            nc.any.tensor_copy(v_sb[:, st, :], ps[:])
        for h in range(1, HPC):
            emit_kn(h)
        ctxd2.close()
        ctxd1.close()

        # q_b outputs (live q_b -> attention); decompression inputs
        ctx_q = ExitStack()
        attbq = ctx_q.enter_context(tc.tile_pool(name="attbq", bufs=1))
        qnope = [attbq.tile([128, S], BF, name=f"qnope{h}") for h in range(HPC)]
        # rope q per head, duplicated into both 64-partition halves so rope
        # score matmuls for adjacent key-tiles can row-pack in the PE array
        qfpe = [attbq.tile([128, S], BF, name=f"qfpe{h}") for h in range(HPC)]

        # ---------------- phase 2: q_b projection (+ q rope) ----------------
        ctx2 = ExitStack()
        sinp = ctx2.enter_context(tc.tile_pool(name="sinp", bufs=1))
        tabq = ctx2.enter_context(tc.tile_pool(name="tabq", bufs=1))
        qrp = ctx2.enter_context(tc.tile_pool(name="qrp", bufs=12))
        qbwp = ctx2.enter_context(tc.tile_pool(name="qbwp", bufs=4))
        ropep = ctx2.enter_context(tc.tile_pool(name="ropep", bufs=2))
        qb_ps = ctx2.enter_context(tc.tile_pool(name="qb_ps", bufs=8, space="PSUM"))

        # sin/cos tables for q rope
        sin_q = sinp.tile([128, S], F, name="t_sinq")
        cos_q = sinp.tile([128, S], F, name="t_cosq")
        posf_row = tabq.tile([1, S], R, name="posf_row")
        pos_i_row = tabq.tile([1, S], I32, name="pos_i_row")
        nc.sync.dma_start(pos_i_row[:], pos_all[:])
        nc.vector.tensor_copy(posf_row[:], pos_i_row[:])
        for cchunk in range(4):
            cs = ts(cchunk, 512)
            bc = qb_ps.tile([128, 512], F, tag="qb_ps", name=f"tabbc{cchunk}")
            nc.tensor.matmul(
                bc[:], ones_row[:], posf_row[:, cs], start=True, stop=True,
            )
            nc.vector.tensor_scalar_mul(sin_q[:, cs], bc[:], if128[:])
            nc.vector.tensor_scalar(cos_q[:, cs], sin_q[:, cs], np.pi / 2.0, None, ALU.add)
            _emit_range_reduce(nc, tabq, sin_q[:, cs], 512)
            _emit_range_reduce(nc, tabq, cos_q[:, cs], 512)
            nc.scalar.activation(sin_q[:, cs], sin_q[:, cs], AF.Sin)
            nc.scalar.activation(cos_q[:, cs], cos_q[:, cs], AF.Sin)

        for sb in range(4):
            qr_tiles = []
            for r in range(NR):
                t = qrp.tile([128, 512], BF, tag="qr", name=f"qr{sb}_{r}")
                g1q_h = g1qA if r < 6 else g1qB
                nc.sync.dma_start(
                    t[:].rearrange("p (c s) -> p c s", c=2),
                    g1q_h.ap()[2 * sb : 2 * sb + 2, ts(r % 6, 128), :]
                    .rearrange("c p s -> p c s"),
                )
                qr_tiles.append(t)
            psj = [
                qb_ps.tile([128, 512], F, tag="qb_ps", name=f"qbps{sb}_{j}")
                for j in range(8)
            ]
            for r in range(NR):
                w = qbwp.tile([128, 1024], BF, tag="qbw", name=f"qbw{sb}_{r}")
                nc.scalar.dma_start(w[:], qbwT.ap()[ts(r, 128), :])
                for j in range(8):
                    nc.tensor.matmul(
                        psj[j][:], w[:, ts(j, 128)], qr_tiles[r][:],
                        start=(r == 0), stop=(r == NR - 1),
                    )
            for h in range(HPC):
                nc.any.tensor_copy(qnope[h][:, ts(sb, 512)], psj[h][:])
            for p in range(2):
                t1 = ropep.tile([128, 512], F, tag="rope1", name=f"rp1_{sb}_{p}")
                t2 = ropep.tile([128, 512], F, tag="rope2", name=f"rp2_{sb}_{p}")
                nc.vector.tensor_tensor(
                    t1[:], psj[4 + p][:], cos_q[:, ts(sb, 512)], ALU.mult
                )
                nc.vector.tensor_tensor(
                    t2[:], psj[6 + p][:], sin_q[:, ts(sb, 512)], ALU.mult
                )
                for hh in range(2):
                    sl = ds(64 * hh, 64)
                    for dh in range(2):
                        nc.vector.tensor_tensor(
                            qfpe[2 * p + hh][ds(64 * dh, 64), ts(sb, 512)],
                            t1[sl, :], t2[sl, :], ALU.add,
                        )
        ctx2.close()

        # prefetch o_proj weights on the scalar queue (3-slot rotation)
        ow_t = []
        for g in range(HPC):
            if g == 0:
                t = owp.tile([128, NCORES, MCOLS], F8, tag="ow8", bufs=1, name="ow8")
                nc.sync.dma_start(t[:], ow8_d.ap())
            else:
                t = owp.tile([128, NCORES, MCOLS], BF, tag="ow", bufs=3, name=f"ow{g}")
                nc.sync.dma_start(t[:], ow_d.ap()[g])
            ow_t.append(t)

        # ---------------- phase 4: attention ----------------
        ctx3 = ExitStack()
        probp = ctx3.enter_context(tc.tile_pool(name="probp", bufs=6))
        dsump = ctx3.enter_context(tc.tile_pool(name="dsump", bufs=3))
        attup = ctx3.enter_context(tc.tile_pool(name="attup", bufs=2))
        attp = ctx3.enter_context(tc.tile_pool(name="attp", bufs=2))
        invp = ctx3.enter_context(tc.tile_pool(name="invp", bufs=2))
        denp = ctx3.enter_context(tc.tile_pool(name="denp", bufs=2))
        sc_ps = ctx3.enter_context(tc.tile_pool(name="sc_ps", bufs=5, space="PSUM"))
        av_ps = ctx3.enter_context(tc.tile_pool(name="av_ps", bufs=2, space="PSUM"))
        den_ps = ctx3.enter_context(tc.tile_pool(name="den_ps", bufs=1, space="PSUM"))

        PIPE = 3  # key-tiles of score-lookahead ahead of exp/av/den consumers
        pending = []  # deferred finalize/trigger emissions

        def flush_pending():
            while pending:
                pending.pop(0)()

        gts = {}

        for h in range(HPC):
            kn = kn_all[h]
            pe_rhs = qfpe[h]
            pe_lhs = kpe_rep
            for qb in range(4):
                nkt = 4 * (qb + 1)
                avt = av_ps.tile([128, 512], F, tag="av", name=f"av{h}_{qb}")
                dent = den_ps.tile([1, 512], F, tag="den", name=f"den{h}_{qb}")
                sct = [None] * nkt
                prob = [None] * nkt
                dsum = [None] * max(qb, 1)

                def emit_nope(kt, h=h, qb=qb, kn=kn, sct=sct):
                    trim = max(0, 128 * (kt - 4 * qb))
                    qsl = ds(512 * qb + trim, 512 - trim)
                    sct[kt] = sc_ps.tile([128, 512], F, tag="sc", name=f"sc{h}{qb}_{kt}")
                    nc.tensor.matmul(
                        sct[kt][:, trim:512], kn[:, ts(kt, 128)], qnope[h][:, qsl],
                        start=True, stop=False,
                    )

                def emit_rope(kt, qb=qb, pe_rhs=pe_rhs, pe_lhs=pe_lhs, sct=sct):
                    # kt parity picks the PE row half -> adjacent pairs pack
                    trim = max(0, 128 * (kt - 4 * qb))
                    qsl = ds(512 * qb + trim, 512 - trim)
                    half = ds(64 * (kt % 2), 64)
                    nc.tensor.matmul(
                        sct[kt][:, trim:512], pe_lhs[half, ts(kt, 128)],
                        pe_rhs[half, qsl],
                        start=False, stop=True,
                    )

                def emit_post(kt, h=h, qb=qb, sct=sct, prob=prob, dsum=dsum):
                    trim = max(0, 128 * (kt - 4 * qb))
                    prob[kt] = probp.tile(
                        [128, 512], BF, tag="prob", name=f"pr{h}{qb}_{kt}"
                    )
                    nc.scalar.activation(
                        prob[kt][:, trim:512], sct[kt][:, trim:512], AF.Exp
                    )
                    if kt >= 4 * qb:
                        nc.vector.tensor_tensor(
                            prob[kt][:, trim : trim + 128],
                            prob[kt][:, trim : trim + 128],
                            triu[:],
                            ALU.mult,
                        )
                    else:
                        # group partial prob sums (full tiles only) so the
                        # denominator needs one matmul per 4 key-tiles
                        g4 = kt // 4
                        if kt % 4 == 1:
                            dsum[g4] = dsump.tile(
                                [128, 512], BF, tag="dsum", name=f"ds{h}{qb}_{g4}"
                            )
                            nc.vector.tensor_tensor(
                                dsum[g4][:], prob[kt - 1][:], prob[kt][:], ALU.add
                            )
                        elif kt % 4 >= 2:
                            nc.vector.tensor_tensor(
                                dsum[g4][:], dsum[g4][:], prob[kt][:], ALU.add
                            )

                def emit_scores(kt):
                    emit_nope(kt)
                    if kt % 2 == 1:
                        emit_rope(kt - 1)
                        emit_rope(kt)
                        emit_post(kt - 1)
                        emit_post(kt)

                def emit_av(kt, h=h, qb=qb, avt=avt, dent=dent, prob=prob,
                            dsum=dsum, nkt=nkt):
                    trim = max(0, 128 * (kt - 4 * qb))
                    nc.tensor.matmul(
                        avt[:, trim:512], v_sb[:, kt, ts(h, 128)],
                        prob[kt][:, trim:512],
                        start=(kt == 0), stop=(kt == nkt - 1),
                    )
                    if kt < 4 * qb:
                        if kt % 4 == 3:
                            nc.tensor.matmul(
                                dent[:], ones_col[:], dsum[kt // 4][:],
                                start=(kt == 3), stop=False,
                            )
                    else:
                        nc.tensor.matmul(
                            dent[:, trim:512], ones_col[:], prob[kt][:, trim:512],
                            start=(kt == 0 and qb == 0), stop=(kt == nkt - 1),
                        )

                for kt in range(min(PIPE, nkt)):
                    emit_scores(kt)
                flush_pending()
                for kt in range(nkt):
                    if kt + PIPE < nkt:
                        emit_scores(kt + PIPE)
                    emit_av(kt)

                den_sb = denp.tile([1, 512], R, tag="densb", name=f"densb{h}_{qb}")
                nc.vector.tensor_copy(den_sb[:], dent[:])
                att_u = attup.tile([128, 512], F, tag="attu", name=f"attu{h}_{qb}")
                nc.vector.tensor_copy(att_u[:], avt[:])

                def finalize(h=h, qb=qb, den_sb=den_sb, att_u=att_u):
                    bc = sc_ps.tile([128, 512], F, tag="sc", name=f"bc{h}_{qb}")
                    row = inv64_row if h == 0 else ones_row
                    nc.tensor.matmul(bc[:], row[:], den_sb[:], start=True, stop=True)
                    inv = invp.tile([128, 512], F, tag="inv", name=f"inv{h}_{qb}")
                    nc.vector.reciprocal_approx_fast(inv[:], bc[:])
                    att = attp.tile(
                        [128, 512], G2DT[h], tag=f"att{h == 0}", name=f"att{h}_{qb}"
                    )
                    nc.vector.tensor_tensor(att[:], att_u[:], inv[:], ALU.mult)
                    nc.gpsimd.dma_start(g2_src[h].ap()[:, ts(qb, 512)], att[:])

                pending.append(finalize)

            def trigger(h=h):
                nc.gpsimd.collective_compute(
                    "AllGather", ALU.bypass,
                    ins=[g2_src[h].ap().opt()], outs=[g2[h].ap().opt()],
                    replica_groups=RG,
                )
                for sblk in range(4):
                    gt = g2p.tile(
                        [128, NCORES, 512], G2DT[h], tag=f"g2t{h == 0}",
                        name=f"g2t{h}_{sblk}"
                    )
                    nc.sync.dma_start(
                        gt[:],
                        g2[h].ap()[:, :, ts(sblk, 512)].rearrange("c p s -> p c s"),
                    )
                    gts[(h, sblk)] = gt

            pending.append(trigger)
        flush_pending()
        ctx3.close()
        ctx_q.close()

        # ---------------- phase 5: o_proj, transposed (out_T[m, s]) ----------------
        ctx5 = ExitStack()
        osb = ctx5.enter_context(tc.tile_pool(name="osb", bufs=1))
        o_ps = ctx5.enter_context(tc.tile_pool(name="o_ps", bufs=4, space="PSUM"))

        oacc = [
            osb.tile([128, 512], F, tag=f"oacc{i}", name=f"oacc{i}")
            for i in range(16)
        ]
        for g in range(HPC):
            for sblk in range(4):
                gt = gts[(g, sblk)]
                for mt in range(MCOLS // 128):
                    ps = o_ps.tile([128, 512], F, tag="o", name=f"ops{g}_{sblk}_{mt}")
                    if g == 0:
                        for r2 in range(NCORES // 2):
                            nc.tensor.matmul(
                                ps[:],
                                ow_t[0][:, ds(2 * r2, 2), ts(mt, 128)],
                                gt[:, ds(2 * r2, 2), :],
                                start=(r2 == 0), stop=(r2 == NCORES // 2 - 1),
                                perf_mode=mybir.MatmulPerfMode.DoubleRow,
                            )
                    else:
                        for r8 in range(NCORES):
                            nc.tensor.matmul(
                                ps[:], ow_t[g][:, r8, ts(mt, 128)], gt[:, r8, :],
                                start=(r8 == 0), stop=(r8 == NCORES - 1),
                            )
                    acc = oacc[sblk * 4 + mt]
                    if g == 0:
                        nc.vector.tensor_scalar_mul(acc[:], ps[:], 1.0 / 512.0)
                    else:
                        nc.vector.tensor_tensor(acc[:], acc[:], ps[:], ALU.add)
                    if g == HPC - 1:
                        nc.gpsimd.dma_start(
                            out_d.ap()[ts(mt, 128), ts(sblk, 512)], acc[:]
                        )
        ctx5.close()

    nc.compile()
    return nc


def _get_nc():
    global _CACHED_NC
    if _CACHED_NC is None:
        _CACHED_NC = _build_program()
    return _CACHED_NC


def _prep_inputs(hidden_states, position_ids, q_a_w, q_a_ln_w, q_b_w, kv_a_w,
                 kv_a_ln_w, kv_b_w, o_w):
    hidden_states = np.asarray(hidden_states, dtype=np.float32)
    position_ids = np.asarray(position_ids, dtype=np.int32)
    q_a_w = np.asarray(q_a_w, dtype=np.float32)
    q_a_ln_w = np.asarray(q_a_ln_w, dtype=np.float32)
    q_b_w = np.asarray(q_b_w, dtype=np.float32)
    kv_a_w = np.asarray(kv_a_w, dtype=np.float32)
    kv_a_ln_w = np.asarray(kv_a_ln_w, dtype=np.float32)
    kv_b_w = np.asarray(kv_b_w, dtype=np.float32)
    o_w = np.asarray(o_w, dtype=np.float32)
    bf = ml_dtypes.bfloat16

    xT_full = np.ascontiguousarray(hidden_states[0].T)          # (H, S)
    qawT = np.ascontiguousarray(q_a_w.T)                         # (H, QR)
    kvawT = np.ascontiguousarray(kv_a_w.T)                       # (H, KR+DR)
    ident = np.eye(128, dtype=np.float32)
    triu = np.triu(np.ones((128, 128), dtype=np.float32))        # keep k <= q
    inv_freq = (1.0 / (THETA ** (np.arange(0, DR, 2) / DR))).astype(np.float32)
    if32 = np.tile(inv_freq[None, :], (128, 1)).astype(np.float32)
    if128 = np.tile(inv_freq, 4)[:, None].astype(np.float32)
    owT_full = np.ascontiguousarray(o_w.T)                       # (N*DV, H)

    # DMA-contiguous weight layouts for the q_a / kv_a streams
    qaw_l = np.empty((12, 128, 8, 512), dtype=bf)
    for ch in range(3):
        for ktg in range(4):
            blk = qawT[ktg * 1024 : (ktg + 1) * 1024, ch * 512 : (ch + 1) * 512]
            qaw_l[ch * 4 + ktg] = blk.reshape(8, 128, 512).transpose(1, 0, 2)
    kvaw_l = np.empty((8, 128, 8, 288), dtype=bf)
    for ch in range(2):
        for ktg in range(4):
            blk = kvawT[ktg * 1024 : (ktg + 1) * 1024, ch * 288 : (ch + 1) * 288]
            kvaw_l[ch * 4 + ktg] = blk.reshape(8, 128, 288).transpose(1, 0, 2)

    in_maps = []
    for c in range(NCORES):
        heads = slice(HPC * c, HPC * (c + 1))
        qb = q_b_w.reshape(N_HEADS, QD, QR)[heads]               # (4, 192, QR)
        nope = qb[:, :DN, :].reshape(HPC * DN, QR)
        pe = qb[:, DN:, :]
        pe_d = np.concatenate([pe[:, 0::2, :], pe[:, 1::2, :]], axis=1)  # (4,64,QR)
        pe_r = np.concatenate([-pe_d[:, 32:, :], pe_d[:, :32, :]], axis=1)
        cols = np.concatenate(
            [nope, pe_d.reshape(HPC * DR, QR), pe_r.reshape(HPC * DR, QR)], axis=0
        )                                                        # (1024, QR)
        qbwT_c = np.ascontiguousarray((cols * (SCALE * q_a_ln_w[None, :])).T)

        kvb = kv_b_w.reshape(N_HEADS, DN + DV, KR)[heads]
        kcols = np.concatenate(
            [kvb[:, :DN, :].reshape(HPC * DN, KR),
             kvb[:, DN:, :].reshape(HPC * DV, KR)],
            axis=0,
        )                                                        # (1024, KR)
        kvbwT_c = np.ascontiguousarray((kcols * kv_a_ln_w[None, :]).T)

        # o_w slice in [g][p][r8][m] layout
        ow_c = owT_full[:, MCOLS * c : MCOLS * (c + 1)]          # (4096, 512)
        ow_l = np.ascontiguousarray(
            ow_c.reshape(NCORES, HPC, 128, MCOLS).transpose(1, 2, 0, 3)
        ).astype(bf)                                             # (4, 128, 8, 512)

        xt_shard = xT_full[:, SL * c : SL * (c + 1)]             # (4096, 256)
        xt_l = np.ascontiguousarray(
            xt_shard.reshape(32, 128, SL).transpose(1, 0, 2)
        ).astype(bf)                                             # (128, 32, 256)

        ow8_l = np.ascontiguousarray(ow_l[0].astype(np.float32) * 8.0).astype(
            ml_dtypes.float8_e4m3fn
        )
        in_maps.append(
            {
                "xt_d": xt_l,
                "ow8_d": ow8_l,
                "qaw_d": qaw_l,
                "kvaw_d": kvaw_l,
                "qbwT": qbwT_c.astype(bf),
                "kvbwT": kvbwT_c.astype(bf),
                "ow_d": ow_l,
                "pos_all": np.ascontiguousarray(position_ids.reshape(1, S)),
                "pos_loc": np.ascontiguousarray(
                    position_ids.reshape(-1)[SL * c : SL * (c + 1)]
                ),
                "ident": ident,
                "triu": triu,
                "if32": if32,
                "if128": if128,
            }
        )
    return in_maps


def kernel(**inputs):
    global LAST_RESULT
    nc = _get_nc()
    in_maps = _prep_inputs(**inputs)
    res = run_bass_kernel_spmd(nc, in_maps, list(range(NCORES)))
    LAST_RESULT = res
    out = np.concatenate([res.results[c]["out"].T for c in range(NCORES)], axis=1)
    return out[None].astype(np.float32)


# revision 43
# speedup vs baseline: 1.0375x; 1.0034x over previous
"""DeepSeekV3 MLA attention prefill kernel for 8 Trainium2 NeuronCores.

Sharding: sequence-parallel for the low-rank input projections (q_a / kv_a),
AllGather of the shared latents, tensor-parallel over heads (4 heads/core) for
q_b / kv_b decompression and attention, AllGather of attention outputs, and
output-column-parallel o_proj (each core produces a disjoint 512-column slice
of the output, concatenated on host).

Schedule notes (final):
- kv_a runs first so AG_kv is the first collective (absorbs the ncfw
  pickup + init-barrier latency); its latents DMA in during q_a
- AG_q is split into two pipelined halves (rt-major transposed writes) so
  q_b can start contracting on the first half while the second transfers
- kv_b decompression emitted between q_a and q_b, filling the AG_q window
- input/weight DMAs spread across sync/scalar/gpsimd issue queues with
  host-side DMA-contiguous weight layouts
- attention software-pipelined 3 key-tiles ahead of the exp chain; softmax
  finalize (den broadcast / fast-approx reciprocal / g2 write) deferred
  behind the next block's matmuls so the PE never drains at boundaries
- o_proj: g2 reads prefetched per head-group as its AllGather lands,
  deep PSUM buffering, SBUF accumulation across head-groups
"""

import sys

sys.path.insert(0, "/opt/trn_rl_repo")

import numpy as np
import ml_dtypes

import concourse.bass as bass  # noqa: F401
import concourse.mybir as mybir
from concourse import bacc
from concourse.bass import ds, ts
from concourse.tile import TileContext
from concourse.bass_utils import run_bass_kernel_spmd
from contextlib import ExitStack

F = mybir.dt.float32
BF = mybir.dt.bfloat16
R = mybir.dt.float32r
I32 = mybir.dt.int32
F8 = mybir.dt.float8e4
AF = mybir.ActivationFunctionType
ALU = mybir.AluOpType

NCORES = 8
B, S, H = 1, 2048, 4096
N_HEADS = 32
HPC = N_HEADS // NCORES          # heads per core = 4
SL = S // NCORES                 # sequence rows per core = 256
QR, KR = 1536, 512
DR, DN, DV = 64, 128, 128
QD = DN + DR                     # 192
SCALE = QD ** -0.5
EPS = 1e-6
THETA = 10000.0
TWO_PI = float(2.0 * np.pi)
MCOLS = H // NCORES              # output columns per core = 512

LAST_RESULT = None               # test harness reads exec_time_ns from here
_CACHED_NC = None
_UID = [0]


def _uid():
    _UID[0] += 1
    return _UID[0]


def _emit_range_reduce(nc, pool, t_ap, width):
    """In-place wrap t_ap (f32, [128, width]) to [-pi, pi]. f32->i32 copy
    rounds to nearest (verified on HW)."""
    tn = pool.tile([128, width], F, tag=f"rr_f_{width}", name=f"rrf{_uid()}")
    ti = pool.tile([128, width], I32, tag=f"rr_i_{width}", name=f"rri{_uid()}")
    nc.vector.tensor_scalar_mul(tn[:], t_ap, 1.0 / TWO_PI)
    nc.vector.tensor_copy(ti[:], tn[:])
    nc.vector.tensor_copy(tn[:], ti[:])
    nc.vector.tensor_scalar_mul(tn[:], tn[:], -TWO_PI)
    nc.vector.tensor_tensor(t_ap, t_ap, tn[:], ALU.add)


def _build_program():
    nc = bacc.Bacc(None, target_bir_lowering=False, num_devices=NCORES)

    # ---------------- DRAM declarations ----------------
    xt_d = nc.dram_tensor("xt_d", [128, 32, SL], BF, kind="ExternalInput")
    qaw_d = nc.dram_tensor("qaw_d", [12, 128, 8, 512], BF, kind="ExternalInput")
    kvaw_d = nc.dram_tensor("kvaw_d", [8, 128, 8, 288], BF, kind="ExternalInput")
    qbwT = nc.dram_tensor("qbwT", [QR, 1024], BF, kind="ExternalInput")
    kvbwT = nc.dram_tensor("kvbwT", [KR, 1024], BF, kind="ExternalInput")
    ow_d = nc.dram_tensor("ow_d", [HPC, 128, NCORES, MCOLS], BF, kind="ExternalInput")
    ow8_d = nc.dram_tensor("ow8_d", [128, NCORES, MCOLS], F8, kind="ExternalInput")
    pos_all = nc.dram_tensor("pos_all", [1, S], I32, kind="ExternalInput")
    pos_loc = nc.dram_tensor("pos_loc", [SL], I32, kind="ExternalInput")
    ident_d = nc.dram_tensor("ident", [128, 128], F, kind="ExternalInput")
    triu_d = nc.dram_tensor("triu", [128, 128], F, kind="ExternalInput")
    if32_d = nc.dram_tensor("if32", [128, 32], F, kind="ExternalInput")
    if128_d = nc.dram_tensor("if128", [128, 1], F, kind="ExternalInput")
    out_d = nc.dram_tensor("out", [MCOLS, S], F, kind="ExternalOutput")

    g1kv_src = nc.dram_tensor("g1kv_src", [KR + DR, SL], BF)
    g1kv = nc.dram_tensor("g1kv", [NCORES, KR + DR, SL], BF, addr_space="Shared")
    QH = QR // 2
    g1q_srcA = nc.dram_tensor("g1q_srcA", [QH, SL], BF)
    g1qA = nc.dram_tensor("g1qA", [NCORES, QH, SL], BF, addr_space="Shared")
    g1q_srcB = nc.dram_tensor("g1q_srcB", [QH, SL], BF)
    g1qB = nc.dram_tensor("g1qB", [NCORES, QH, SL], BF, addr_space="Shared")
    warm_src = nc.dram_tensor("warm_src", [1, 128], BF)
    warm_dst = nc.dram_tensor("warm_dst", [NCORES, 1, 128], BF, addr_space="Shared")
    G2DT = [F8, BF, BF, BF]
    g2_src = [nc.dram_tensor(f"g2src{h}", [DV, S], G2DT[h]) for h in range(HPC)]
    g2 = [
        nc.dram_tensor(f"g2_{h}", [NCORES, DV, S], G2DT[h], addr_space="Shared")
        for h in range(HPC)
    ]
    RG = [list(range(NCORES))]
    NKT = H // 128  # 32 k-tiles over the model dim
    NR = QR // 128  # 12 k-tiles over q_lora_rank
    NKR = KR // 128  # 4 k-tiles over kv_lora_rank

    with TileContext(nc) as tc, ExitStack() as ctx:
        persist = ctx.enter_context(tc.tile_pool(name="persist", bufs=1))

        # ---------------- constants ----------------
        ident = persist.tile([128, 128], F, name="c_ident")
        nc.gpsimd.dma_start(ident[:], ident_d[:])
        triu_f = persist.tile([128, 128], F, name="c_triu_f")
        nc.gpsimd.dma_start(triu_f[:], triu_d[:])
        triu = persist.tile([128, 128], BF, name="c_triu")
        nc.vector.tensor_copy(triu[:], triu_f[:])
        if32 = persist.tile([128, 32], F, name="c_if32")
        nc.gpsimd.dma_start(if32[:], if32_d[:])
        if128 = persist.tile([128, 1], F, name="c_if128")
        nc.gpsimd.dma_start(if128[:], if128_d[:])
        ones_f = persist.tile([128, 1], F, name="c_ones_f")
        nc.vector.memset(ones_f[:], 1.0)
        ones_fr = persist.tile([1, 128], F, name="c_ones_fr")
        nc.vector.memset(ones_fr[:], 1.0)
        ones_col = persist.tile([128, 1], BF, name="c_ones_col")
        nc.vector.tensor_copy(ones_col[:], ones_f[:])
        ones_row = persist.tile([1, 128], R, name="c_ones_row")
        nc.vector.tensor_copy(ones_row[:], ones_fr[:])
        inv64_fr = persist.tile([1, 128], F, name="c_inv64_fr")
        nc.vector.memset(inv64_fr[:], 1.0 / 64.0)
        inv64_row = persist.tile([1, 128], R, name="c_inv64_row")
        nc.vector.tensor_copy(inv64_row[:], inv64_fr[:])
        eps_t = persist.tile([128, 1], F, name="c_eps")
        nc.vector.memset(eps_t[:], EPS)
        sin_k = [persist.tile([128, 32], F, name=f"t_sink{st}") for st in range(2)]
        cos_k = [persist.tile([128, 32], F, name=f"t_cosk{st}") for st in range(2)]

        # long-lived pools (stack-bottom): attention latents + o_proj staging
        kpe_p = ctx.enter_context(tc.tile_pool(name="kpe_p", bufs=1))
        vp = ctx.enter_context(tc.tile_pool(name="vp", bufs=1))
        knall = ctx.enter_context(tc.tile_pool(name="knall", bufs=1))
        owp = ctx.enter_context(tc.tile_pool(name="owp", bufs=1))
        g2p = ctx.enter_context(tc.tile_pool(name="g2p", bufs=2))

        kpe_rep = kpe_p.tile([128, S], BF, name="kpe_rep")
        v_sb = vp.tile([128, S // 128, 512], BF, name="v_sb")
        kn_all = [knall.tile([128, S], BF, name=f"kn{h}") for h in range(HPC)]

        # k_pe rope tables (local positions); scratch lives in persist so the
        # pool stack has no boundary here (xt DMAs must not wait on this)
        posf_loc = persist.tile([128, 2], F, name="posf_loc")
        pos_i_loc = persist.tile([128, 2], I32, name="pos_i_loc")
        nc.gpsimd.dma_start(
            pos_i_loc[:], pos_loc.ap().rearrange("(t p) -> p t", p=128)
        )
        nc.vector.tensor_copy(posf_loc[:], pos_i_loc[:])
        for st in range(2):
            nc.vector.tensor_scalar_mul(
                sin_k[st][:], if32[:], posf_loc[:, st : st + 1]
            )
            nc.vector.tensor_scalar(
                cos_k[st][:], sin_k[st][:], np.pi / 2.0, None, ALU.add
            )
            _emit_range_reduce(nc, persist, sin_k[st][:], 32)
            _emit_range_reduce(nc, persist, cos_k[st][:], 32)
            nc.scalar.activation(sin_k[st][:], sin_k[st][:], AF.Sin)
            nc.scalar.activation(cos_k[st][:], cos_k[st][:], AF.Sin)

        ctxd1 = ExitStack()
        ckvp = ctxd1.enter_context(tc.tile_pool(name="ckvp", bufs=1))

        # ---------------- phase 0: q_a then kv_a projections ----------------
        ctx0 = ExitStack()
        xtp = ctx0.enter_context(tc.tile_pool(name="xtp", bufs=1))
        wp0 = ctx0.enter_context(tc.tile_pool(name="wp0", bufs=3))
        p0 = ctx0.enter_context(tc.tile_pool(name="p0", bufs=2))

        # x shard: chunks 0/1 first (every matmul needs xt[kt0] immediately);
        # chunks 2/3 queue behind the first kv weight transfers
        xt = xtp.tile([128, NKT, SL], BF, name="xt")
        for q4 in (0, 1):
            eng = nc.sync if q4 % 2 == 0 else nc.scalar
            eng.dma_start(xt[:, ds(8 * q4, 8), :], xt_d.ap()[:, ds(8 * q4, 8), :])

        # gpsimd is free until its first collective trigger -> 3 DMA queues
        DMA_ROT = [nc.sync, nc.scalar, nc.gpsimd]

        # --- kv_a (chunks of 288 cols) ---
        ctx0a = ExitStack()
        kv_ps = ctx0a.enter_context(tc.tile_pool(name="kv_ps", bufs=4, space="PSUM"))
        tr_ps = ctx0a.enter_context(tc.tile_pool(name="tr_ps", bufs=2, space="PSUM"))
        trk_ps = ctx0a.enter_context(tc.tile_pool(name="trk_ps", bufs=1, space="PSUM"))
        trsb = ctx0a.enter_context(tc.tile_pool(name="trsb", bufs=3))

        kvch = [[None] * 2 for _ in range(2)]
        for ch in range(2):
            for st in range(2):
                kvch[st][ch] = kv_ps.tile(
                    [128, 288], F, tag="kv_ps", name=f"kvps{st}_{ch}"
                )
        for ch in range(2):
            for ktg in range(NKT // 8):
                w = wp0.tile([128, 8, 288], BF, tag="kvw", name=f"kvw{ch}_{ktg}")
                DMA_ROT[(ch * 4 + ktg) % 3].dma_start(w[:], kvaw_d.ap()[ch * 4 + ktg])
                if ch == 0 and ktg == 1:
                    for q4 in (2, 3):
                        eng = nc.sync if q4 % 2 == 0 else nc.scalar
                        eng.dma_start(
                            xt[:, ds(8 * q4, 8), :], xt_d.ap()[:, ds(8 * q4, 8), :]
                        )
                for kk in range(8):
                    kt = ktg * 8 + kk
                    for st in range(2):
                        nc.tensor.matmul(
                            kvch[st][ch][:], xt[:, kt, ts(st, 128)], w[:, kk],
                            start=(kt == 0), stop=(kt == NKT - 1),
                        )
        for st in range(2):
            acc0 = p0.tile([128, 1], F, tag="kvacc", name=f"kvacc0_{st}")
            acc1 = p0.tile([128, 1], F, tag="kvacc", name=f"kvacc1_{st}")
            scr = p0.tile([128, 288], F, tag="kvscr", name=f"kvscr{st}")
            nc.scalar.activation(scr[:], kvch[st][0][:], AF.Square, accum_out=acc0[:])
            nc.scalar.activation(
                scr[:, 0:224], kvch[st][1][:, 0:224], AF.Square, accum_out=acc1[:]
            )
            nc.vector.tensor_tensor(acc0[:], acc0[:], acc1[:], ALU.add)
            stdv = p0.tile([128, 1], F, tag="kvstd", name=f"kvstd{st}")
            nc.scalar.activation(stdv[:], acc0[:], AF.Sqrt, bias=eps_t[:], scale=1.0 / KR)
            rinv = p0.tile([128, 1], F, tag="kvrinv", name=f"kvrinv{st}")
            nc.vector.reciprocal(rinv[:], stdv[:])
            ckvn = p0.tile([128, KR], F, tag="ckvn", name=f"ckvn{st}")
            nc.vector.tensor_scalar_mul(ckvn[:, 0:288], kvch[st][0][:], rinv[:])
            nc.vector.tensor_scalar_mul(ckvn[:, 288:512], kvch[st][1][:, 0:224], rinv[:])
            # rope k_pe: cols 512:576 of kv_a = chunk1 cols 224:288, deinterleaved
            pe = kvch[st][1][:, 224:288].rearrange("p (d two) -> p two d", two=2)
            y1, y2 = pe[:, 0], pe[:, 1]
            kr_t = p0.tile([128, DR], F, tag="kr", name=f"kr{st}")
            t1 = p0.tile([128, 32], F, tag="krt1", name=f"krt1_{st}")
            t2 = p0.tile([128, 32], F, tag="krt2", name=f"krt2_{st}")
            nc.vector.tensor_tensor(t1[:], y1, cos_k[st][:], ALU.mult)
            nc.vector.tensor_tensor(t2[:], y2, sin_k[st][:], ALU.mult)
            nc.vector.tensor_tensor(kr_t[:, 0:32], t1[:], t2[:], ALU.subtract)
            nc.vector.tensor_tensor(t1[:], y2, cos_k[st][:], ALU.mult)
            nc.vector.tensor_tensor(t2[:], y1, sin_k[st][:], ALU.mult)
            nc.vector.tensor_tensor(kr_t[:, 32:64], t1[:], t2[:], ALU.add)
            for rt in range(KR // 128):
                tp = tr_ps.tile([128, 128], F, tag="tr", name=f"kvtr{st}_{rt}")
                nc.tensor.transpose(tp[:], ckvn[:, ts(rt, 128)], ident[:])
                sb_t = trsb.tile([128, 128], BF, tag="trsb", name=f"kvtrs{st}_{rt}")
                nc.any.tensor_copy(sb_t[:], tp[:])
                nc.sync.dma_start(g1kv_src.ap()[ts(rt, 128), ts(st, 128)], sb_t[:])
            tpk = trk_ps.tile([64, 128], F, tag="trk", name=f"kvtrk{st}")
            nc.tensor.transpose(tpk[:], kr_t[:], ident[:])
            sb_k = trsb.tile([64, 128], BF, tag="trsbk", name=f"kvtrks{st}")
            nc.any.tensor_copy(sb_k[:], tpk[:])
            nc.sync.dma_start(g1kv_src.ap()[KR : KR + DR, ts(st, 128)], sb_k[:])
        ctx0a.close()

        nc.gpsimd.collective_compute(
            "AllGather", ALU.bypass,
            ins=[g1kv_src.ap().opt()], outs=[g1kv.ap().opt()], replica_groups=RG,
        )

        # decompression inputs: issue on gpsimd right after the AG_kv trigger
        # so the transfers run during q_b compute
        for half in range(2):
            nc.gpsimd.dma_start(
                kpe_rep[ds(64 * half, 64), :].rearrange("p (c s) -> p c s", c=NCORES),
                g1kv.ap()[:, KR : KR + DR, :].rearrange("c p s -> p c s"),
            )
        ckv_t = []
        for r in range(NKR):
            t = ckvp.tile([128, S], BF, name=f"ckv{r}")
            nc.gpsimd.dma_start(
                t[:].rearrange("p (c s) -> p c s", c=NCORES),
                g1kv.ap()[:, ts(r, 128), :].rearrange("c p s -> p c s"),
            )
            ckv_t.append(t)
        kvbv, kvbn = [], []
        for r in range(NKR):
            tv = ckvp.tile([128, 512], BF, name=f"kvbv{r}")
            nc.gpsimd.dma_start(tv[:], kvbwT.ap()[ts(r, 128), 512:1024])
            kvbv.append(tv)
            tn = ckvp.tile([128, 512], BF, name=f"kvbn{r}")
            nc.gpsimd.dma_start(tn[:], kvbwT.ap()[ts(r, 128), 0:512])
            kvbn.append(tn)

        # --- q_a (chunks of 512 cols) ---
        ctx0b = ExitStack()
        qa_ps = ctx0b.enter_context(tc.tile_pool(name="qa_ps", bufs=6, space="PSUM"))
        tr2_ps = ctx0b.enter_context(tc.tile_pool(name="tr2_ps", bufs=2, space="PSUM"))
        tr2sb = ctx0b.enter_context(tc.tile_pool(name="tr2sb", bufs=3))
        qch = [[None] * 3 for _ in range(2)]
        for ch in range(3):
            for st in range(2):
                qch[st][ch] = qa_ps.tile(
                    [128, 512], F, tag="qa_ps", name=f"qaps{st}_{ch}"
                )
        for ch in range(3):
            for ktg in range(NKT // 8):
                w = wp0.tile([128, 8, 512], BF, tag="qaw", name=f"qaw{ch}_{ktg}")
                idx = ch * 4 + ktg
                (nc.sync if idx % 2 == 0 else nc.scalar).dma_start(
                    w[:], qaw_d.ap()[idx]
                )
                for kk in range(8):
                    kt = ktg * 8 + kk
                    for st in range(2):
                        nc.tensor.matmul(
                            qch[st][ch][:], xt[:, kt, ts(st, 128)], w[:, kk],
                            start=(kt == 0), stop=(kt == NKT - 1),
                        )
        qn_t = []
        for st in range(2):
            accs = []
            scr = p0.tile([128, 512], F, tag="qascr", name=f"qascr{st}")
            for ch in range(3):
                a = p0.tile([128, 1], F, tag="qaacc", name=f"qaacc{st}_{ch}")
                nc.scalar.activation(scr[:], qch[st][ch][:], AF.Square, accum_out=a[:])
                accs.append(a)
            nc.vector.tensor_tensor(accs[0][:], accs[0][:], accs[1][:], ALU.add)
            nc.vector.tensor_tensor(accs[0][:], accs[0][:], accs[2][:], ALU.add)
            stdv = p0.tile([128, 1], F, tag="qastd", name=f"qastd{st}")
            nc.scalar.activation(stdv[:], accs[0][:], AF.Sqrt, bias=eps_t[:], scale=1.0 / QR)
            rinv = p0.tile([128, 1], F, tag="qarinv", name=f"qarinv{st}")
            nc.vector.reciprocal(rinv[:], stdv[:])
            qn = p0.tile([128, QR], F, tag="qn", name=f"qn{st}", bufs=2)
            for ch in range(3):
                nc.vector.tensor_scalar_mul(qn[:, ts(ch, 512)], qch[st][ch][:], rinv[:])
            qn_t.append(qn)
        # rt-major transposed writes: half A (rt 0-5) completes early so its
        # AllGather pipelines ahead of half B
        for half, g1q_src in ((0, g1q_srcA), (1, g1q_srcB)):
            for st in range(2):
                for rt6 in range(6):
                    rt = 6 * half + rt6
                    tp = tr2_ps.tile([128, 128], F, tag="tr2", name=f"qtr{st}_{rt}")
                    nc.tensor.transpose(tp[:], qn_t[st][:, ts(rt, 128)], ident[:])
                    sb_t = tr2sb.tile([128, 128], BF, tag="tr2sb", name=f"qtrs{st}_{rt}")
                    nc.any.tensor_copy(sb_t[:], tp[:])
                    nc.sync.dma_start(
                        g1q_src.ap()[ts(rt6, 128), ts(st, 128)], sb_t[:]
                    )
            nc.gpsimd.collective_compute(
                "AllGather", ALU.bypass,
                ins=[(g1q_srcA if half == 0 else g1q_srcB).ap().opt()],
                outs=[(g1qA if half == 0 else g1qB).ap().opt()],
                replica_groups=RG,
            )
        ctx0b.close()

        ctx0.close()

        # ---------------- decompression (kn for h=0 first, then v, rest) ----
        ctxd2 = ExitStack()
        dec_ps = ctxd2.enter_context(tc.tile_pool(name="dec_ps", bufs=4, space="PSUM"))

        def emit_kn(h):
            for sb in range(4):
                ps = dec_ps.tile([128, 512], F, tag="dec", name=f"knps{h}_{sb}")
                for r in range(NKR):
                    nc.tensor.matmul(
                        ps[:], kvbn[r][:, ts(h, 128)], ckv_t[r][:, ts(sb, 512)],
                        start=(r == 0), stop=(r == NKR - 1),
                    )
                nc.any.tensor_copy(kn_all[h][:, ts(sb, 512)], ps[:])

        emit_kn(0)
        for st in range(S // 128):
            ps = dec_ps.tile([128, 512], F, tag="dec", name=f"vps{st}")
            for r in range(NKR):
                nc.tensor.matmul(
                    ps[:], ckv_t[r][:, ts(st, 128)], kvbv[r][:],
                    start=(r == 0), stop=(r == NKR - 1),
                )
            nc.any.tensor_copy(v_sb[:, st, :], ps[:])
        for h in range(1, HPC):
            emit_kn(h)
        ctxd2.close()
        ctxd1.close()

        # q_b outputs (live q_b -> attention); decompression inputs
        ctx_q = ExitStack()
        attbq = ctx_q.enter_context(tc.tile_pool(name="attbq", bufs=1))
        qnope = [attbq.tile([128, S], BF, name=f"qnope{h}") for h in range(HPC)]
        # rope q per head, duplicated into both 64-partition halves so rope
        # score matmuls for adjacent key-tiles can row-pack in the PE array
        qfpe = [attbq.tile([128, S], BF, name=f"qfpe{h}") for h in range(HPC)]

        # ---------------- phase 2: q_b projection (+ q rope) ----------------
        ctx2 = ExitStack()
        sinp = ctx2.enter_context(tc.tile_pool(name="sinp", bufs=1))
        tabq = ctx2.enter_context(tc.tile_pool(name="tabq", bufs=1))
        qrp = ctx2.enter_context(tc.tile_pool(name="qrp", bufs=12))
        qbwp = ctx2.enter_context(tc.tile_pool(name="qbwp", bufs=4))
        ropep = ctx2.enter_context(tc.tile_pool(name="ropep", bufs=2))
        qb_ps = ctx2.enter_context(tc.tile_pool(name="qb_ps", bufs=8, space="PSUM"))

        # sin/cos tables for q rope
        sin_q = sinp.tile([128, S], F, name="t_sinq")
        cos_q = sinp.tile([128, S], F, name="t_cosq")
        posf_row = tabq.tile([1, S], R, name="posf_row")
        pos_i_row = tabq.tile([1, S], I32, name="pos_i_row")
        nc.sync.dma_start(pos_i_row[:], pos_all[:])
        nc.vector.tensor_copy(posf_row[:], pos_i_row[:])
        for cchunk in range(4):
            cs = ts(cchunk, 512)
            bc = qb_ps.tile([128, 512], F, tag="qb_ps", name=f"tabbc{cchunk}")
            nc.tensor.matmul(
                bc[:], ones_row[:], posf_row[:, cs], start=True, stop=True,
            )
            nc.vector.tensor_scalar_mul(sin_q[:, cs], bc[:], if128[:])
            nc.vector.tensor_scalar(cos_q[:, cs], sin_q[:, cs], np.pi / 2.0, None, ALU.add)
            _emit_range_reduce(nc, tabq, sin_q[:, cs], 512)
            _emit_range_reduce(nc, tabq, cos_q[:, cs], 512)
            nc.scalar.activation(sin_q[:, cs], sin_q[:, cs], AF.Sin)
            nc.scalar.activation(cos_q[:, cs], cos_q[:, cs], AF.Sin)

        for sb in range(4):
            qr_tiles = []
            for r in range(NR):
                t = qrp.tile([128, 512], BF, tag="qr", name=f"qr{sb}_{r}")
                g1q_h = g1qA if r < 6 else g1qB
                nc.sync.dma_start(
                    t[:].rearrange("p (c s) -> p c s", c=2),
                    g1q_h.ap()[2 * sb : 2 * sb + 2, ts(r % 6, 128), :]
                    .rearrange("c p s -> p c s"),
                )
                qr_tiles.append(t)
            psj = [
                qb_ps.tile([128, 512], F, tag="qb_ps", name=f"qbps{sb}_{j}")
                for j in range(8)
            ]
            for r in range(NR):
                w = qbwp.tile([128, 1024], BF, tag="qbw", name=f"qbw{sb}_{r}")
                nc.scalar.dma_start(w[:], qbwT.ap()[ts(r, 128), :])
                for j in range(8):
                    nc.tensor.matmul(
                        psj[j][:], w[:, ts(j, 128)], qr_tiles[r][:],
                        start=(r == 0), stop=(r == NR - 1),
                    )
            for h in range(HPC):
                nc.any.tensor_copy(qnope[h][:, ts(sb, 512)], psj[h][:])
            for p in range(2):
                t1 = ropep.tile([128, 512], F, tag="rope1", name=f"rp1_{sb}_{p}")
                t2 = ropep.tile([128, 512], F, tag="rope2", name=f"rp2_{sb}_{p}")
                nc.vector.tensor_tensor(
                    t1[:], psj[4 + p][:], cos_q[:, ts(sb, 512)], ALU.mult
                )
                nc.vector.tensor_tensor(
                    t2[:], psj[6 + p][:], sin_q[:, ts(sb, 512)], ALU.mult
                )
                for hh in range(2):
                    sl = ds(64 * hh, 64)
                    for dh in range(2):
                        nc.vector.tensor_tensor(
                            qfpe[2 * p + hh][ds(64 * dh, 64), ts(sb, 512)],
                            t1[sl, :], t2[sl, :], ALU.add,
                        )
        ctx2.close()

        # prefetch o_proj weights on the scalar queue (3-slot rotation)
        ow_t = []
        for g in range(HPC):
            if g == 0:
                t = owp.tile([128, NCORES, MCOLS], F8, tag="ow8", bufs=1, name="ow8")
                nc.sync.dma_start(t[:], ow8_d.ap())
            else:
                t = owp.tile([128, NCORES, MCOLS], BF, tag="ow", bufs=3, name=f"ow{g}")
                nc.sync.dma_start(t[:], ow_d.ap()[g])
            ow_t.append(t)

        # ---------------- phase 4: attention ----------------
        ctx3 = ExitStack()
        probp = ctx3.enter_context(tc.tile_pool(name="probp", bufs=6))
        dsump = ctx3.enter_context(tc.tile_pool(name="dsump", bufs=3))
        attup = ctx3.enter_context(tc.tile_pool(name="attup", bufs=2))
        attp = ctx3.enter_context(tc.tile_pool(name="attp", bufs=2))
        invp = ctx3.enter_context(tc.tile_pool(name="invp", bufs=2))
        denp = ctx3.enter_context(tc.tile_pool(name="denp", bufs=2))
        sc_ps = ctx3.enter_context(tc.tile_pool(name="sc_ps", bufs=5, space="PSUM"))
        av_ps = ctx3.enter_context(tc.tile_pool(name="av_ps", bufs=2, space="PSUM"))
        den_ps = ctx3.enter_context(tc.tile_pool(name="den_ps", bufs=1, space="PSUM"))

        PIPE = 3  # key-tiles of score-lookahead ahead of exp/av/den consumers
        pending = []  # deferred finalize/trigger emissions

        def flush_pending():
            while pending:
                pending.pop(0)()

        gts = {}

        for h in range(HPC):
            kn = kn_all[h]
            pe_rhs = qfpe[h]
            pe_lhs = kpe_rep
            for qb in range(4):
                nkt = 4 * (qb + 1)
                avt = av_ps.tile([128, 512], F, tag="av", name=f"av{h}_{qb}")
                dent = den_ps.tile([1, 512], F, tag="den", name=f"den{h}_{qb}")
                sct = [None] * nkt
                prob = [None] * nkt
                dsum = [None] * max(qb, 1)

                def emit_nope(kt, h=h, qb=qb, kn=kn, sct=sct):
                    trim = max(0, 128 * (kt - 4 * qb))
                    qsl = ds(512 * qb + trim, 512 - trim)
                    sct[kt] = sc_ps.tile([128, 512], F, tag="sc", name=f"sc{h}{qb}_{kt}")
                    nc.tensor.matmul(
                        sct[kt][:, trim:512], kn[:, ts(kt, 128)], qnope[h][:, qsl],
                        start=True, stop=False,
                    )

                def emit_rope(kt, qb=qb, pe_rhs=pe_rhs, pe_lhs=pe_lhs, sct=sct):
                    # kt parity picks the PE row half -> adjacent pairs pack
                    trim = max(0, 128 * (kt - 4 * qb))
                    qsl = ds(512 * qb + trim, 512 - trim)
                    half = ds(64 * (kt % 2), 64)
                    nc.tensor.matmul(
                        sct[kt][:, trim:512], pe_lhs[half, ts(kt, 128)],
                        pe_rhs[half, qsl],
                        start=False, stop=True,
                    )

                def emit_post(kt, h=h, qb=qb, sct=sct, prob=prob, dsum=dsum):
                    trim = max(0, 128 * (kt - 4 * qb))
                    prob[kt] = probp.tile(
                        [128, 512], BF, tag="prob", name=f"pr{h}{qb}_{kt}"
                    )
                    nc.scalar.activation(
                        prob[kt][:, trim:512], sct[kt][:, trim:512], AF.Exp
                    )
                    if kt >= 4 * qb:
                        nc.vector.tensor_tensor(
                            prob[kt][:, trim : trim + 128],
                            prob[kt][:, trim : trim + 128],
                            triu[:],
                            ALU.mult,
                        )
                    else:
                        # group partial prob sums (full tiles only) so the
                        # denominator needs one matmul per 4 key-tiles
                        g4 = kt // 4
                        if kt % 4 == 1:
                            dsum[g4] = dsump.tile(
                                [128, 512], BF, tag="dsum", name=f"ds{h}{qb}_{g4}"
                            )
                            nc.vector.tensor_tensor(
                                dsum[g4][:], prob[kt - 1][:], prob[kt][:], ALU.add
                            )
                        elif kt % 4 >= 2:
                            nc.vector.tensor_tensor(
                                dsum[g4][:], dsum[g4][:], prob[kt][:], ALU.add
                            )

                def emit_scores(kt):
                    emit_nope(kt)
                    if kt % 2 == 1:
                        emit_rope(kt - 1)
                        emit_rope(kt)
                        emit_post(kt - 1)
                        emit_post(kt)

                def emit_av(kt, h=h, qb=qb, avt=avt, dent=dent, prob=prob,
                            dsum=dsum, nkt=nkt):
                    trim = max(0, 128 * (kt - 4 * qb))
                    nc.tensor.matmul(
                        avt[:, trim:512], v_sb[:, kt, ts(h, 128)],
                        prob[kt][:, trim:512],
                        start=(kt == 0), stop=(kt == nkt - 1),
                    )
                    if kt < 4 * qb:
                        if kt % 4 == 3:
                            nc.tensor.matmul(
                                dent[:], ones_col[:], dsum[kt // 4][:],
                                start=(kt == 3), stop=False,
                            )
                    else:
                        nc.tensor.matmul(
                            dent[:, trim:512], ones_col[:], prob[kt][:, trim:512],
                            start=(kt == 0 and qb == 0), stop=(kt == nkt - 1),
                        )

                for kt in range(min(PIPE, nkt)):
                    emit_scores(kt)
                flush_pending()
                for kt in range(nkt):
                    if kt + PIPE < nkt:
                        emit_scores(kt + PIPE)
                    emit_av(kt)

                den_sb = denp.tile([1, 512], R, tag="densb", name=f"densb{h}_{qb}")
                nc.vector.tensor_copy(den_sb[:], dent[:])
                att_u = attup.tile([128, 512], F, tag="attu", name=f"attu{h}_{qb}")
                nc.vector.tensor_copy(att_u[:], avt[:])

                def finalize(h=h, qb=qb, den_sb=den_sb, att_u=att_u):
                    bc = sc_ps.tile([128, 512], F, tag="sc", name=f"bc{h}_{qb}")
                    row = inv64_row if h == 0 else ones_row
                    nc.tensor.matmul(bc[:], row[:], den_sb[:], start=True, stop=True)
                    inv = invp.tile([128, 512], F, tag="inv", name=f"inv{h}_{qb}")
                    nc.vector.reciprocal_approx_fast(inv[:], bc[:])
                    att = attp.tile(
                        [128, 512], G2DT[h], tag=f"att{h == 0}", name=f"att{h}_{qb}"
                    )
                    nc.vector.tensor_tensor(att[:], att_u[:], inv[:], ALU.mult)
                    nc.gpsimd.dma_start(g2_src[h].ap()[:, ts(qb, 512)], att[:])

                pending.append(finalize)

            def trigger(h=h):
                nc.gpsimd.collective_compute(
                    "AllGather", ALU.bypass,
                    ins=[g2_src[h].ap().opt()], outs=[g2[h].ap().opt()],
                    replica_groups=RG,
                )
                for sblk in range(4):
                    gt = g2p.tile(
                        [128, NCORES, 512], G2DT[h], tag=f"g2t{h == 0}",
                        name=f"g2t{h}_{sblk}"
                    )
                    nc.sync.dma_start(
                        gt[:],
                        g2[h].ap()[:, :, ts(sblk, 512)].rearrange("c p s -> p c s"),
                    )
                    gts[(h, sblk)] = gt

            pending.append(trigger)
        flush_pending()
        ctx3.close()
        ctx_q.close()

        # ---------------- phase 5: o_proj, transposed (out_T[m, s]) ----------------
        ctx5 = ExitStack()
        osb = ctx5.enter_context(tc.tile_pool(name="osb", bufs=1))
        o_ps = ctx5.enter_context(tc.tile_pool(name="o_ps", bufs=4, space="PSUM"))

        oacc = [
            osb.tile([128, 512], F, tag=f"oacc{i}", name=f"oacc{i}")
            for i in range(16)
        ]
        for g in range(HPC):
            for sblk in range(4):
                gt = gts[(g, sblk)]
                for mt in range(MCOLS // 128):
                    ps = o_ps.tile([128, 512], F, tag="o", name=f"ops{g}_{sblk}_{mt}")
                    if g == 0:
                        for r2 in range(NCORES // 2):
                            nc.tensor.matmul(
                                ps[:],
                                ow_t[0][:, ds(2 * r2, 2), ts(mt, 128)],
                                gt[:, ds(2 * r2, 2), :],
                                start=(r2 == 0), stop=(r2 == NCORES // 2 - 1),
                                perf_mode=mybir.MatmulPerfMode.DoubleRow,
                            )
                    else:
                        for r8 in range(NCORES):
                            nc.tensor.matmul(
                                ps[:], ow_t[g][:, r8, ts(mt, 128)], gt[:, r8, :],
                                start=(r8 == 0), stop=(r8 == NCORES - 1),
                            )
                    acc = oacc[sblk * 4 + mt]
                    if g == 0:
                        nc.vector.tensor_scalar_mul(acc[:], ps[:], 1.0 / 512.0)
                    else:
                        nc.vector.tensor_tensor(acc[:], acc[:], ps[:], ALU.add)
                    if g == HPC - 1:
                        nc.gpsimd.dma_start(
                            out_d.ap()[ts(mt, 128), ts(sblk, 512)], acc[:]
                        )
        ctx5.close()

    nc.compile()
    return nc


def _get_nc():
    global _CACHED_NC
    if _CACHED_NC is None:
        _CACHED_NC = _build_program()
    return _CACHED_NC


def _prep_inputs(hidden_states, position_ids, q_a_w, q_a_ln_w, q_b_w, kv_a_w,
                 kv_a_ln_w, kv_b_w, o_w):
    hidden_states = np.asarray(hidden_states, dtype=np.float32)
    position_ids = np.asarray(position_ids, dtype=np.int32)
    q_a_w = np.asarray(q_a_w, dtype=np.float32)
    q_a_ln_w = np.asarray(q_a_ln_w, dtype=np.float32)
    q_b_w = np.asarray(q_b_w, dtype=np.float32)
    kv_a_w = np.asarray(kv_a_w, dtype=np.float32)
    kv_a_ln_w = np.asarray(kv_a_ln_w, dtype=np.float32)
    kv_b_w = np.asarray(kv_b_w, dtype=np.float32)
    o_w = np.asarray(o_w, dtype=np.float32)
    bf = ml_dtypes.bfloat16

    xT_full = np.ascontiguousarray(hidden_states[0].T)          # (H, S)
    qawT = np.ascontiguousarray(q_a_w.T)                         # (H, QR)
    kvawT = np.ascontiguousarray(kv_a_w.T)                       # (H, KR+DR)
    ident = np.eye(128, dtype=np.float32)
    triu = np.triu(np.ones((128, 128), dtype=np.float32))        # keep k <= q
    inv_freq = (1.0 / (THETA ** (np.arange(0, DR, 2) / DR))).astype(np.float32)
    if32 = np.tile(inv_freq[None, :], (128, 1)).astype(np.float32)
    if128 = np.tile(inv_freq, 4)[:, None].astype(np.float32)
    owT_full = np.ascontiguousarray(o_w.T)                       # (N*DV, H)

    # DMA-contiguous weight layouts for the q_a / kv_a streams
    qaw_l = np.empty((12, 128, 8, 512), dtype=bf)
    for ch in range(3):
        for ktg in range(4):
            blk = qawT[ktg * 1024 : (ktg + 1) * 1024, ch * 512 : (ch + 1) * 512]
            qaw_l[ch * 4 + ktg] = blk.reshape(8, 128, 512).transpose(1, 0, 2)
    kvaw_l = np.empty((8, 128, 8, 288), dtype=bf)
    for ch in range(2):
        for ktg in range(4):
            blk = kvawT[ktg * 1024 : (ktg + 1) * 1024, ch * 288 : (ch + 1) * 288]
            kvaw_l[ch * 4 + ktg] = blk.reshape(8, 128, 288).transpose(1, 0, 2)

    in_maps = []
    for c in range(NCORES):
        heads = slice(HPC * c, HPC * (c + 1))
        qb = q_b_w.reshape(N_HEADS, QD, QR)[heads]               # (4, 192, QR)
        nope = qb[:, :DN, :].reshape(HPC * DN, QR)
        pe = qb[:, DN:, :]
        pe_d = np.concatenate([pe[:, 0::2, :], pe[:, 1::2, :]], axis=1)  # (4,64,QR)
        pe_r = np.concatenate([-pe_d[:, 32:, :], pe_d[:, :32, :]], axis=1)
        cols = np.concatenate(
            [nope, pe_d.reshape(HPC * DR, QR), pe_r.reshape(HPC * DR, QR)], axis=0
        )                                                        # (1024, QR)
        qbwT_c = np.ascontiguousarray((cols * (SCALE * q_a_ln_w[None, :])).T)

        kvb = kv_b_w.reshape(N_HEADS, DN + DV, KR)[heads]
        kcols = np.concatenate(
            [kvb[:, :DN, :].reshape(HPC * DN, KR),
             kvb[:, DN:, :].reshape(HPC * DV, KR)],
            axis=0,
        )                                                        # (1024, KR)
        kvbwT_c = np.ascontiguousarray((kcols * kv_a_ln_w[None, :]).T)

        # o_w slice in [g][p][r8][m] layout
        ow_c = owT_full[:, MCOLS * c : MCOLS * (c + 1)]          # (4096, 512)
        ow_l = np.ascontiguousarray(
            ow_c.reshape(NCORES, HPC, 128, MCOLS).transpose(1, 2, 0, 3)
        ).astype(bf)                                             # (4, 128, 8, 512)

        xt_shard = xT_full[:, SL * c : SL * (c + 1)]             # (4096, 256)
        xt_l = np.ascontiguousarray(
            xt_shard.reshape(32, 128, SL).transpose(1, 0, 2)
        ).astype(bf)                                             # (128, 32, 256)

        ow8_l = np.ascontiguousarray(ow_l[0].astype(np.float32) * 8.0).astype(
            ml_dtypes.float8_e4m3fn
        )
        in_maps.append(
            {
                "xt_d": xt_l,
                "ow8_d": ow8_l,
                "qaw_d": qaw_l,
                "kvaw_d": kvaw_l,
                "qbwT": qbwT_c.astype(bf),
                "kvbwT": kvbwT_c.astype(bf),
                "ow_d": ow_l,
                "pos_all": np.ascontiguousarray(position_ids.reshape(1, S)),
                "pos_loc": np.ascontiguousarray(
                    position_ids.reshape(-1)[SL * c : SL * (c + 1)]
                ),
                "ident": ident,
                "triu": triu,
                "if32": if32,
                "if128": if128,
            }
        )
    return in_maps


def kernel(**inputs):
    global LAST_RESULT
    nc = _get_nc()
    in_maps = _prep_inputs(**inputs)
    res = run_bass_kernel_spmd(nc, in_maps, list(range(NCORES)))
    LAST_RESULT = res
    out = np.concatenate([res.results[c]["out"].T for c in range(NCORES)], axis=1)
    return out[None].astype(np.float32)
